# revision 1
# baseline (speedup 1.0000x reference)
"""Trainium2 Bass kernel for nn_CGLSTM (TwoStageFusion + 3-layer gamma-modulated LSTM).

Sharding: pure data parallel over batch B=256 across 8 NeuronCores (32 per core).
Per-core device program (all matmuls f32r = TF32-like, full-rate):
  Phase A: |hrrp| transposed to feature-major DRAM scratch via PE transpose.
  Phase B: fusion (gamma) computed per batch row as a feature-major matmul
           chain; written time-shifted (gamma_sh) into an SBUF-resident buffer.
  Phase C: 3-layer LSTM as a layer wavefront: iteration s processes layer 0
           at t=s, layer 1 at t=s-1, layer 2 at t=s-2.  All gate matmuls
           accumulate into one [96, 512] PSUM tile (batch-stationary with
           zero-padded lhsT blocks per layer; f32r matmuls must write PSUM
           partition 0), elementwise batched across the three layers, h
           transposed back to feature-major by PE each iteration.
"""

import os
import sys

sys.path.insert(0, "/opt/trn_rl_repo")

import numpy as np  # noqa: E402

import concourse.bass as bass  # noqa: E402, F401
import concourse.tile as tile  # noqa: E402
from concourse import bacc, mybir  # noqa: E402
from concourse.bass_utils import run_bass_kernel_spmd  # noqa: E402

f32 = mybir.dt.float32
f32r = mybir.dt.float32r
ACTF = mybir.ActivationFunctionType
ALU = mybir.AluOpType

B, T, D, H, F = 256, 512, 200, 128, 128
LAM = 0.5
NCORES = 8
BS = B // NCORES  # 32
DA, DB = 128, D - 128  # x feature chunks


def _R(t):
    """f32r view of a whole tile (bitcast first, slice after)."""
    return t[:].bitcast(f32r)


def build_nc(t_steps=T):
    TS = t_steps
    nc = bacc.Bacc("TRN2", target_bir_lowering=False, debug=False, num_devices=NCORES)

    def dt_in(name, shape):
        return nc.dram_tensor(name, shape, f32, kind="ExternalInput").ap()

    hrrp = dt_in("hrrp", [BS * TS, D])
    ac = nc.dram_tensor("ac", [BS, TS], f32r, kind="ExternalInput").ap()
    pc = nc.dram_tensor("pc", [BS, TS], f32r, kind="ExternalInput").ap()
    rldel = nc.dram_tensor("rldel", [BS, TS, 2], f32r, kind="ExternalInput").ap()
    w0h = dt_in("w0h", [H, 4 * H])
    w0xa = dt_in("w0xa", [DA, 4 * H])
    gw0 = dt_in("gw0", [F, 4 * H])
    w1h = dt_in("w1h", [H, 4 * H])
    w1x = dt_in("w1x", [H, 4 * H])
    gw1 = dt_in("gw1", [F, 4 * H])
    w2h = dt_in("w2h", [H, 4 * H])
    w2x = dt_in("w2x", [H, 4 * H])
    gw2 = dt_in("gw2", [F, 4 * H])
    w0xbb = dt_in("w0xbb", [DB + 1, 4 * H])
    bias12 = dt_in("bias12", [2, 4 * H])
    fw_amp = dt_in("fw_amp", [1, F])
    fw_ph = dt_in("fw_ph", [1, F])
    fw_gate = dt_in("fw_gate", [2 * F, F])
    fw_r1 = dt_in("fw_r1", [2, F])
    fw_r2 = dt_in("fw_r2", [F, F])
    fw_q = dt_in("fw_q", [F, F])
    fw_k = dt_in("fw_k", [F, F])
    fw_vo = dt_in("fw_vo", [F, F])
    brow_q = dt_in("brow_q", [1, F])
    bc_k = dt_in("bc_k", [F, 1])
    bc_vo = dt_in("bc_vo", [F, 1])
    bc_amp = dt_in("bc_amp", [F, 1])
    bc_ph = dt_in("bc_ph", [F, 1])
    bc_gate = dt_in("bc_gate", [F, 1])
    bc_r1 = dt_in("bc_r1", [F, 1])
    bc_r2 = dt_in("bc_r2", [F, 1])
    bc_out = dt_in("bc_out", [F, 1])
    identd = dt_in("identd", [128, 128])
    onesrow = dt_in("onesrow", [1, max(TS, 512)])
    onescol = dt_in("onescol", [128, 1])
    regw = dt_in("regw", [H, 2])
    regb = dt_in("regb", [1, 2])
    zz = dt_in("zz", [128, 96])

    outd = nc.dram_tensor("out", [BS, 2], f32, kind="ExternalOutput").ap()

    # DRAM scratch for |x| feature-major, pre-rounded to f32r: [d, t, b]
    xscr_a = nc.dram_tensor("xscr_a", [DA, TS, BS], f32r).ap()
    xscr_b = nc.dram_tensor("xscr_b", [DB, TS, BS], f32r).ap()

    NG = 4 * H
    I0, F0, O0, C0 = 0, H, 2 * H, 3 * H  # gate col offsets, order [i f o c]

    with tile.TileContext(nc) as tc:
        with tc.tile_pool(name="const", bufs=1) as cp:
            def load_r(name, shape, src):
                t = cp.tile(shape, f32, tag=name)
                nc.gpsimd.dma_start(_R(t), src)
                return t

            def load_f(name, shape, src):
                t = cp.tile(shape, f32, tag=name)
                nc.sync.dma_start(t[:], src)
                return t

            s_w0h = load_r("s_w0h", [H, NG], w0h[:])
            s_w0xa = load_r("s_w0xa", [DA, NG], w0xa[:])
            s_gw0 = load_r("s_gw0", [F, NG], gw0[:])
            s_w1h = load_r("s_w1h", [H, NG], w1h[:])
            s_w1x = load_r("s_w1x", [H, NG], w1x[:])
            s_gw1 = load_r("s_gw1", [F, NG], gw1[:])
            s_w2h = load_r("s_w2h", [H, NG], w2h[:])
            s_w2x = load_r("s_w2x", [H, NG], w2x[:])
            s_gw2 = load_r("s_gw2", [F, NG], gw2[:])
            s_w0xbb = load_r("s_w0xbb", [DB + 1, NG], w0xbb[:])
            s_biasr1 = load_r("s_biasr1", [1, NG], bias12[0:1, :])
            s_biasr2 = load_r("s_biasr2", [1, NG], bias12[1:2, :])
            s_fw_amp = load_r("s_fw_amp", [1, F], fw_amp[:])
            s_fw_ph = load_r("s_fw_ph", [1, F], fw_ph[:])
            s_fwg_ph = load_r("s_fwg_ph", [F, F], fw_gate[0:F, :])
            s_fwg_am = load_r("s_fwg_am", [F, F], fw_gate[F:2 * F, :])
            s_fw_r1 = load_r("s_fw_r1", [2, F], fw_r1[:])
            s_fw_r2 = load_r("s_fw_r2", [F, F], fw_r2[:])
            s_fw_q = load_r("s_fw_q", [F, F], fw_q[:])
            s_fw_k = load_r("s_fw_k", [F, F], fw_k[:])
            s_fw_vo = load_r("s_fw_vo", [F, F], fw_vo[:])
            s_bq = load_r("s_bq", [1, F], brow_q[:])
            s_bc_k = load_f("s_bc_k", [F, 1], bc_k[:])
            s_bc_vo = load_f("s_bc_vo", [F, 1], bc_vo[:])
            s_ident = load_f("s_ident", [128, 128], identd[:])
            s_ones = load_r("s_ones", [1, max(TS, 512)], onesrow[:])
            s_onescol = load_r("s_onescol", [128, 1], onescol[:])
            s_regw = load_r("s_regw", [H, 2], regw[:])
            s_regb = load_r("s_regb", [1, 2], regb[:])
            s_bc_amp = load_f("s_bc_amp", [F, 1], bc_amp[:])
            s_bc_ph = load_f("s_bc_ph", [F, 1], bc_ph[:])
            s_bc_gate = load_f("s_bc_gate", [F, 1], bc_gate[:])
            s_bc_r1 = load_f("s_bc_r1", [F, 1], bc_r1[:])
            s_bc_r2 = load_f("s_bc_r2", [F, 1], bc_r2[:])
            s_bc_out = load_f("s_bc_out", [F, 1], bc_out[:])

            # gamma_sh buffer, SBUF resident, layout [F, t, b] (t-major)
            gbuf = cp.tile([F, TS, BS], f32, tag="gbuf")
            nc.gpsimd.dma_start(_R(gbuf)[:, 0, :], zz[:, 0:BS])

            # ---------- phase A: |x| transpose pre-pass ----------
            # each tile holds rows (t-major, b-minor): TB t x 32 b; emitted
            # interleaved with the recurrence (tile k feeds iterations
            # 4k..4k+4), sharing the recurrence transpose PSUM slots.
            TB = 128 // BS  # t's per tile (4)
            hr3 = hrrp.rearrange("(b tt) d -> b tt d", b=BS)
            def emit_prepass_tile(k, pa, pap):
                t0_ = k * TB
                raw = pa.tile([128, D], f32, tag="raw", name=f"raw_{k}")
                nc.sync.dma_start(
                    raw[:],
                    hr3[:, t0_:t0_ + TB, :].rearrange("b tt d -> tt b d"))
                ab = pa.tile([128, D], f32, tag="ab", name=f"ab_{k}")
                nc.vector.tensor_scalar(
                    ab[:].bitcast(mybir.dt.uint32), raw[:].bitcast(mybir.dt.uint32),
                    0x7FFFFFFF, None, ALU.bitwise_and)
                pt1 = pap.tile([128, 128], f32, tag="ptx", name=f"pt1_{k}")
                nc.tensor.transpose(pt1[0:DA, :], ab[:, 0:DA], s_ident[:, :])
                pt2 = pap.tile([128, 128], f32, tag="ptx", name=f"pt2_{k}")
                nc.tensor.transpose(pt2[0:DB, :], ab[:, DA:D], s_ident[:, :])
                sb1 = pa.tile([DA, 128], f32, tag="sb1", name=f"sb1_{k}")
                nc.scalar.activation(_R(sb1), pt1[0:DA, :], ACTF.Copy)
                sb2 = pa.tile([DB, 128], f32, tag="sb2", name=f"sb2_{k}")
                nc.vector.tensor_copy(_R(sb2), pt2[0:DB, :])
                nc.sync.dma_start(xscr_a[:, t0_:t0_ + TB, :], _R(sb1))
                nc.sync.dma_start(xscr_b[:, t0_:t0_ + TB, :], _R(sb2))

            # ---------- phase B: fusion (t-major chunks, interleaved) ----------
            SC = float(F) ** -0.5
            FT = min(16, TS)  # t's per fusion chunk

            def emit_fusion_chunk(j, fu, fup):
                tj = j * FT
                N = FT * BS
                a_row = fu.tile([1, N], f32, tag="a_row", name=f"a_row_{j}")
                nc.sync.dma_start(
                    a_row[:].bitcast(f32r).rearrange("p (tt b) -> p tt b", tt=FT),
                    bass.AP(tensor=ac.tensor, offset=tj,
                            ap=[[0, 1], [1, FT], [TS, BS]]))
                p_row = fu.tile([1, N], f32, tag="p_row", name=f"p_row_{j}")
                nc.sync.dma_start(
                    p_row[:].bitcast(f32r).rearrange("p (tt b) -> p tt b", tt=FT),
                    bass.AP(tensor=pc.tensor, offset=tj,
                            ap=[[0, 1], [1, FT], [TS, BS]]))
                rl2 = fu.tile([2, N], f32, tag="rl2", name=f"rl2_{j}")
                for c_ in range(2):
                    nc.sync.dma_start(
                        rl2[:].bitcast(f32r)[c_:c_ + 1, :].rearrange(
                            "p (tt b) -> p tt b", tt=FT),
                        bass.AP(tensor=rldel.tensor, offset=tj * 2 + c_,
                                ap=[[0, 1], [2, FT], [2 * TS, BS]]))

                pA = fup.tile([F, N], f32, tag="fps", name=f"pA_{j}")
                nc.tensor.matmul(pA[:], _R(s_fw_amp), _R(a_row), start=True, stop=True)
                ampT = fu.tile([F, N], f32, tag="ampT", name=f"ampT_{j}")
                nc.scalar.activation(_R(ampT), pA[:], ACTF.Tanh, bias=s_bc_amp[:])

                pB = fup.tile([F, N], f32, tag="fps", name=f"pB_{j}")
                nc.tensor.matmul(pB[:], _R(s_fw_ph), _R(p_row), start=True, stop=True)
                phT = fu.tile([F, N], f32, tag="phT", name=f"phT_{j}")
                nc.scalar.activation(_R(phT), pB[:], ACTF.Tanh, bias=s_bc_ph[:])

                pC = fup.tile([F, N], f32, tag="fps", name=f"pC_{j}")
                nc.tensor.matmul(pC[:], _R(s_fwg_ph), _R(phT), start=True, stop=False)
                nc.tensor.matmul(pC[:], _R(s_fwg_am), _R(ampT), start=False, stop=True)
                betaT = fu.tile([F, N], f32, tag="betaT", name=f"betaT_{j}")
                nc.scalar.activation(betaT[:], pC[:], ACTF.Sigmoid, bias=s_bc_gate[:])

                dT = fu.tile([F, N], f32, tag="dT", name=f"dT_{j}")
                nc.vector.tensor_tensor(dT[:], phT[:], ampT[:], ALU.subtract)
                mT = fu.tile([F, N], f32, tag="mT", name=f"mT_{j}")
                nc.vector.tensor_tensor(mT[:], betaT[:], dT[:], ALU.mult)
                corrT = fu.tile([F, N], f32, tag="corrT", name=f"corrT_{j}")
                nc.vector.tensor_tensor(_R(corrT), mT[:], ampT[:], ALU.add)

                pR1 = fup.tile([F, N], f32, tag="fps", name=f"pR1_{j}")
                nc.tensor.matmul(pR1[:], _R(s_fw_r1), _R(rl2), start=True, stop=True)
                rl1T = fu.tile([F, N], f32, tag="rl1T", name=f"rl1T_{j}")
                nc.scalar.activation(_R(rl1T), pR1[:], ACTF.Tanh, bias=s_bc_r1[:])
                pR2 = fup.tile([F, N], f32, tag="fps", name=f"pR2_{j}")
                nc.tensor.matmul(pR2[:], _R(s_fw_r2), _R(rl1T), start=True, stop=True)
                rlT = fu.tile([F, N], f32, tag="rlT", name=f"rlT_{j}")
                nc.scalar.activation(_R(rlT), pR2[:], ACTF.Tanh, bias=s_bc_r2[:])

                pQ = fup.tile([F, N], f32, tag="fps", name=f"pQ_{j}")
                nc.tensor.matmul(pQ[:], _R(s_fw_q), _R(corrT), start=True, stop=False)
                nc.tensor.matmul(pQ[:], _R(s_bq), _R(s_ones)[:, 0:N], start=False, stop=True)
                pK = fup.tile([F, N], f32, tag="fps", name=f"pK_{j}")
                nc.tensor.matmul(pK[:], _R(s_fw_k), _R(rlT), start=True, stop=True)
                kT = fu.tile([F, N], f32, tag="kT", name=f"kT_{j}")
                nc.vector.tensor_scalar(kT[:], pK[:], s_bc_k[:], None, ALU.add)

                qkT = fu.tile([F, N], f32, tag="qkT", name=f"qkT_{j}")
                nc.vector.tensor_tensor(_R(qkT), pQ[:], kT[:], ALU.mult)
                pS = fup.tile([1, N], f32, tag="fps", name=f"pS_{j}")
                nc.tensor.matmul(pS[:], _R(s_onescol), _R(qkT), start=True, stop=True)
                attnT = fu.tile([1, N], f32, tag="attnT", name=f"attnT_{j}")
                nc.scalar.activation(_R(attnT), pS[:], ACTF.Sigmoid, scale=SC)

                pG = fup.tile([F, N], f32, tag="fps", name=f"pG_{j}")
                nc.tensor.matmul(pG[:], _R(s_fw_vo), _R(rlT), start=True, stop=True)
                gT = fu.tile([F, N], f32, tag="gT", name=f"gT_{j}")
                nc.vector.tensor_scalar(gT[:], pG[:], s_bc_vo[:], None, ALU.add)
                pBC = fup.tile([F, N], f32, tag="fps", name=f"pBC_{j}")
                nc.tensor.matmul(pBC[:], _R(s_ones)[:, 0:F], _R(attnT), start=True, stop=True)

                tmpT = fu.tile([F, N], f32, tag="tmpT", name=f"tmpT_{j}")
                nc.vector.tensor_tensor(tmpT[:], pBC[:], gT[:], ALU.mult)
                nrow = min(FT, TS - 1 - tj)
                nc.vector.tensor_scalar(
                    _R(gbuf)[:, tj + 1:tj + 1 + nrow, :], tmpT[:, 0:nrow * BS],
                    s_bc_out[:], None, ALU.add)

            # ---------- phase C: recurrence ----------
            # Wavefront: layer l at iteration s processes t = s - l.
            # Three fully decoupled per-layer pipelines; each has its own
            # [32, 512] PSUM z tile (f32r matmuls must write partition 0).
            with (
                tc.tile_pool(name="rc_state", bufs=1) as st,
                tc.tile_pool(name="rc_sb", bufs=2) as rs,
                tc.tile_pool(name="rc_z", bufs=2, space="PSUM") as zp,
                tc.tile_pool(name="rc_pt", bufs=1, space="PSUM") as tp,
                tc.tile_pool(name="rc_fps", bufs=2, space="PSUM") as fps_pool,
            ):
                hT = []
                cst = []
                for l in range(3):
                    pair = []
                    for i in range(2):
                        t = st.tile([128, 32], f32, tag=f"hT{l}_{i}", name=f"hT{l}_{i}")
                        nc.gpsimd.dma_start(_R(t), zz[:, 0:32])
                        pair.append(t)
                    hT.append(pair)
                    cpair = []
                    for i in range(2):
                        t = st.tile([32, H], f32, tag=f"cst{l}_{i}", name=f"cst{l}_{i}")
                        nc.vector.memset(t[:], 0.0)
                        cpair.append(t)
                    cst.append(cpair)

                xa_t = []
                xb_t = []
                for i in range(4):
                    t = st.tile([DA, 32], f32, tag=f"xa_{i}", name=f"xa_{i}")
                    xa_t.append(t)
                    t = st.tile([DB + 1, 32], f32, tag=f"xb_{i}", name=f"xb_{i}")
                    nc.gpsimd.dma_start(_R(t)[DB:DB + 1, :], onesrow[:, 0:32])
                    xb_t.append(t)

                gbr = _R(gbuf)  # [F, TS, BS] f32r view
                W_h = [s_w0h, s_w1h, s_w2h]
                W_x = [None, s_w1x, s_w2x]
                GW = [s_gw0, s_gw1, s_gw2]
                G3 = 3 * H

                NPT = TS // TB  # prepass tiles
                PROLOG = min(8, NPT)
                NFC = (TS + FT - 1) // FT  # fusion chunks
                PROLOG_F = min(3, NFC)
                for k in range(PROLOG):
                    emit_prepass_tile(k, rs, tp)
                for j in range(PROLOG_F):
                    emit_fusion_chunk(j, rs, fps_pool)
                for s in range(TS + 2):
                    pv, nx = s % 2, (s + 1) % 2
                    if s % TB == 0:
                        k = s // TB + PROLOG
                        if k < NPT:
                            emit_prepass_tile(k, rs, tp)
                    if s % FT == FT // 2:
                        j = s // FT + PROLOG_F
                        if j < NFC:
                            emit_fusion_chunk(j, rs, fps_pool)
                    for l in range(3):
                        t_l = s - l
                        if not (0 <= t_l < TS):
                            continue
                        ztag = "z0" if l == 0 else "z12"
                        z = zp.tile([32, NG], f32, tag=ztag, name=f"z{l}_{s}",
                                    bufs=(2 if l == 0 else 3))
                        if l == 0:
                            xa = xa_t[s % 4]
                            nc.sync.dma_start(_R(xa), xscr_a[:, t_l, :])
                            xb = xb_t[s % 4]
                            nc.sync.dma_start(_R(xb)[0:DB, :], xscr_b[:, t_l, :])
                            nc.tensor.matmul(z[:], _R(xa), _R(s_w0xa), start=True, stop=False)
                            nc.tensor.matmul(z[:], _R(xb), _R(s_w0xbb), start=False, stop=False)
                            nc.tensor.matmul(z[:], gbr[:, t_l, :], _R(s_gw0), start=False, stop=False)
                            nc.tensor.matmul(z[:], _R(hT[0][pv]), _R(s_w0h), start=False, stop=True)
                        else:
                            nc.tensor.matmul(z[:], _R(s_ones)[:, 0:32],
                                             _R(s_biasr1 if l == 1 else s_biasr2),
                                             start=True, stop=False)
                            nc.tensor.matmul(z[:], gbr[:, t_l, :], _R(GW[l]), start=False, stop=False)
                            nc.tensor.matmul(z[:], _R(hT[l - 1][pv]), _R(W_x[l]), start=False, stop=False)
                            nc.tensor.matmul(z[:], _R(hT[l][pv]), _R(W_h[l]), start=False, stop=True)

                        sg = rs.tile([32, NG], f32, tag=f"sg{l}", name=f"sg{l}_{s}")
                        nc.scalar.activation(sg[:], z[:], ACTF.Sigmoid)
                        ct = rs.tile([32, H], f32, tag=f"ct{l}", name=f"ct{l}_{s}", bufs=3)
                        nc.vector.tensor_scalar(ct[:], sg[:, C0:C0 + H], 2.0, 1.0,
                                                ALU.mult, ALU.subtract)

                        tfirst = False
                        if tfirst:
                            # o-gate transposed early (off the critical path)
                            sot = tp.tile([128, 32], f32, tag="ptx", name=f"sot{l}_{s}")
                            nc.tensor.transpose(sot[:], sg[:, 2 * H:G3], s_ident[0:32, 0:32])
                            sos = rs.tile([128, 32], f32, tag=f"sos{l}", name=f"sos{l}_{s}")
                            nc.vector.tensor_copy(sos[:], sot[:])

                        m2 = rs.tile([32, H], f32, tag=f"m2{l}", name=f"m2{l}_{s}", bufs=3)
                        nc.vector.tensor_tensor(m2[:], sg[:, H:2 * H], cst[l][pv][:], ALU.mult)
                        m1 = rs.tile([32, H], f32, tag=f"m1{l}", name=f"m1{l}_{s}", bufs=3)
                        nc.vector.tensor_tensor(m1[:], sg[:, 0:H], ct[:], ALU.mult)
                        c_new = cst[l][nx]
                        nc.vector.tensor_tensor(c_new[:], m1[:], m2[:], ALU.add)

                        th = rs.tile([32, H], f32, tag=f"th{l}", name=f"th{l}_{s}", bufs=3)
                        nc.scalar.activation(th[:], c_new[:], ACTF.Tanh)
                        if tfirst:
                            tht = tp.tile([128, 32], f32, tag="ptx", name=f"tht{l}_{s}")
                            nc.tensor.transpose(tht[:], th[:], s_ident[0:32, 0:32])
                            # h^T = sigma(o)^T * tanh(c2)^T straight into hT
                            nc.vector.tensor_tensor(_R(hT[l][nx]), sos[:], tht[:], ALU.mult)
                        else:
                            hb = rs.tile([32, H], f32, tag=f"hb{l}", name=f"hb{l}_{s}", bufs=3)
                            nc.vector.tensor_tensor(hb[:], sg[:, 2 * H:G3], th[:], ALU.mult)
                            pt = tp.tile([128, 32], f32, tag="ptx", name=f"pt{l}_{s}")
                            nc.tensor.transpose(pt[:], hb[:], s_ident[0:32, 0:32])
                            nc.vector.tensor_copy(_R(hT[l][nx]), pt[:])

                last = (TS + 2) % 2
                po = tp.tile([32, 2], f32, tag="ptx", name="po")
                nc.tensor.matmul(po[:], _R(hT[2][last]), _R(s_regw), start=True, stop=False)
                nc.tensor.matmul(po[:], _R(s_ones)[:, 0:32], _R(s_regb), start=False, stop=True)
                outs = rs.tile([32, 2], f32, tag="outs")
                nc.scalar.copy(outs[:], po[:])
                nc.sync.dma_start(outd[:], outs[:])

    nc.compile()
    return nc


def prep_inputs(inputs, t_steps=T):
    TS = t_steps

    def g(k):
        return np.asarray(inputs[k], dtype=np.float32)

    # gate col permutation [i f c o] -> [i f o c]
    perm = np.concatenate([np.arange(0, H), np.arange(H, 2 * H),
                           np.arange(3 * H, 4 * H), np.arange(2 * H, 3 * H)])

    base_w0 = g("base_w0")[:, perm]
    w0h = np.ascontiguousarray(base_w0[:H])
    w0x = base_w0[H:]
    w0xa = np.ascontiguousarray(w0x[:DA])
    w0xb = np.ascontiguousarray(w0x[DA:])
    b0 = g("base_b0")[perm]
    bw12 = g("base_w12")
    w1 = bw12[0][:, perm]
    w2 = bw12[1][:, perm]
    w1h, w1x = np.ascontiguousarray(w1[:H]), np.ascontiguousarray(w1[H:])
    w2h, w2x = np.ascontiguousarray(w2[:H]), np.ascontiguousarray(w2[H:])
    b12 = g("base_b12")
    b1, b2 = b12[0][perm], b12[1][perm]

    def gwstack(gw):  # [4, F, H] -> [F, 4H] cols [i f o c], lambda folded
        return np.concatenate([gw[0], -LAM * gw[1], gw[3], gw[2]], axis=1)

    gw0 = gwstack(g("gam_w0"))
    gw12 = g("gam_w12")
    gw1, gw2 = gwstack(gw12[0]), gwstack(gw12[1])

    w0xbb = np.concatenate([w0xb, b0[None, :]], axis=0).astype(np.float32)
    bias12 = np.stack([b1, b2]).astype(np.float32)
    # tanh(x) = 2*sigmoid(2x)-1: pre-scale the c-gate columns by 2 so one
    # full-width sigmoid covers all four gates
    for _arr in (w0h, w0xa, w0xbb, gw0, w1h, w1x, gw1, w2h, w2x, gw2, bias12):
        _arr[..., 3 * H:4 * H] *= 2.0

    f_v_w, f_out_w = g("f_v_w"), g("f_out_w")
    f_v_b, f_out_b = g("f_v_b"), g("f_out_b")
    fw_vo = (f_v_w @ f_out_w).astype(np.float32)
    b_vo = (f_v_b @ f_out_w).astype(np.float32)

    consts = {
        "w0h": w0h, "w0xa": w0xa, "w0xbb": w0xbb, "gw0": gw0,
        "w1h": w1h, "w1x": w1x, "gw1": gw1,
        "w2h": w2h, "w2x": w2x, "gw2": gw2,
        "bias12": bias12,
        "fw_amp": g("f_amp_w"), "fw_ph": g("f_ph_w"), "fw_gate": g("f_gate_w"),
        "fw_r1": g("f_rlos_w1"), "fw_r2": g("f_rlos_w2"),
        "fw_q": g("f_q_w"), "fw_k": g("f_k_w"), "fw_vo": fw_vo,
        "brow_q": g("f_q_b")[None, :], "bc_k": g("f_k_b")[:, None],
        "bc_vo": b_vo[:, None],
        "bc_amp": g("f_amp_b")[:, None], "bc_ph": g("f_ph_b")[:, None],
        "bc_gate": g("f_gate_b")[:, None], "bc_r1": g("f_rlos_b1")[:, None],
        "bc_r2": g("f_rlos_b2")[:, None], "bc_out": f_out_b[:, None],
        "identd": np.eye(128, dtype=np.float32),
        "onesrow": np.ones((1, max(TS, 512)), np.float32),
        "onescol": np.ones((128, 1), np.float32),
        "regw": g("reg_w"), "regb": g("reg_b")[None, :],
        "zz": np.zeros((128, 96), np.float32),
    }
    consts = {k: np.ascontiguousarray(v, dtype=np.float32) for k, v in consts.items()}

    hrrp = g("hrrp")[:, :TS, :]
    ac = g("amplitude_corr")[:, :TS]
    pc_ = g("phase_corr")[:, :TS]
    rldel = g("rlos_delta")[:, :TS, :]

    in_maps = []
    for c in range(NCORES):
        sl = slice(c * BS, (c + 1) * BS)
        m = dict(consts)
        m["hrrp"] = np.ascontiguousarray(hrrp[sl].reshape(BS * TS, D))
        m["ac"] = np.ascontiguousarray(ac[sl])
        m["pc"] = np.ascontiguousarray(pc_[sl])
        m["rldel"] = np.ascontiguousarray(rldel[sl])
        in_maps.append(m)
    return in_maps


_NC_CACHE = {}


def _get_nc(t_steps=T):
    if t_steps not in _NC_CACHE:
        _NC_CACHE[t_steps] = build_nc(t_steps)
    return _NC_CACHE[t_steps]


def run(inputs, t_steps=T, **kwargs):
    nc = _get_nc(t_steps)
    in_maps = prep_inputs(inputs, t_steps)
    res = run_bass_kernel_spmd(nc, in_maps, core_ids=list(range(NCORES)), **kwargs)
    out = np.concatenate([res.results[c]["out"] for c in range(NCORES)], axis=0)
    return out, res


def kernel(**inputs) -> np.ndarray:
    out, _ = run(inputs)
    return out.astype(np.float32)



# revision 12
# speedup vs baseline: 1.0251x; 1.0251x over previous
"""Trainium2 Bass kernel for nn_CGLSTM (TwoStageFusion + 3-layer gamma-modulated LSTM).

v2: transposed (feature-major) bf16 recurrence.

Sharding: pure data parallel over batch B=256 across 8 NeuronCores (32/core),
split into 2 interleaved groups of 16 to pipeline the serial recurrence.

Per step s and group g, one PSUM tile [128, 240] holds z^T for all 3 wavefront
layers (12 gate-blocks x 16 batch, gate cols pre-scaled so tanh folds into
sigmoid) plus a 48-col region where the *other* group's 2*c state was written
by the vector engines; ONE sigmoid instruction then yields all gates and
tanh(c_other) at once.  Gate matmuls are bf16 with the weights stationary
(out free size 16), x^T and gamma^T live SBUF-resident, so the inner loop
does no DMA and no transposes.
"""

import sys

sys.path.insert(0, "/opt/trn_rl_repo")

import numpy as np  # noqa: E402
import ml_dtypes  # noqa: E402

import concourse.bass as bass  # noqa: E402, F401
import concourse.tile as tile  # noqa: E402
from concourse import bacc, mybir  # noqa: E402
from concourse.bass_utils import run_bass_kernel_spmd  # noqa: E402

f32 = mybir.dt.float32
f32r = mybir.dt.float32r
bf16 = mybir.dt.bfloat16
u32 = mybir.dt.uint32
ACTF = mybir.ActivationFunctionType
ALU = mybir.AluOpType
BF = ml_dtypes.bfloat16

B, T, D, H, F = 256, 512, 200, 128, 128
LAM = 0.5
NCORES = 8
BS = B // NCORES   # 32
G = BS // 2        # 16 per group
DA, DB = 128, D - 128
NG = 4 * H
FT = 16            # fusion chunk t-size (= #stages)
TB = 4             # prepass t's per [128,200] tile
DEBUG_GBUF = False
DEBUG_S = 0
C2O = 192          # c2 region offset in the z psum tile
ZW = C2O + 48      # 240


def _R(t):
    return t[:].bitcast(f32r)


def build_nc(t_steps=T):
    TS = t_steps
    STOT = TS + 2
    nc = bacc.Bacc("TRN2", target_bir_lowering=False, debug=False, num_devices=NCORES)

    def din(name, shape, dt=f32):
        return nc.dram_tensor(name, shape, dt, kind="ExternalInput").ap()

    hrrp = din("hrrp", [BS * TS, D])
    ac = nc.dram_tensor("ac", [BS, TS], f32r, kind="ExternalInput").ap()
    pc = nc.dram_tensor("pc", [BS, TS], f32r, kind="ExternalInput").ap()
    rldel = nc.dram_tensor("rldel", [BS, TS, 2], f32r, kind="ExternalInput").ap()
    # recurrence weights (bf16, gate order [i f o c], c-cols pre-scaled x2)
    w0xa = din("w0xa", [DA, NG], bf16)
    w0xbb = din("w0xbb", [DB + 1, NG], bf16)   # row DB = L0 bias
    w0h = din("w0h", [H, NG], bf16)
    gw0 = din("gw0", [F, NG], bf16)
    w1x = din("w1x", [H, NG], bf16)
    w1h = din("w1h", [H, NG], bf16)
    gw1 = din("gw1", [F, NG], bf16)
    w2x = din("w2x", [H, NG], bf16)
    w2h = din("w2h", [H, NG], bf16)
    gw2 = din("gw2", [F, NG], bf16)
    bstk = din("bstk", [8, 128], bf16)         # L1/L2 biases stacked
    bind = din("bind", [8, ZW], bf16)          # bias block indicator (zero-padded)
    # fusion weights
    fw_amp = din("fw_amp", [1, F])
    fw_ph = din("fw_ph", [1, F])
    fw_r1 = din("fw_r1", [2, F])
    fwg_ph = din("fwg_ph", [F, F], bf16)
    fwg_am = din("fwg_am", [F, F], bf16)
    fw_r2 = din("fw_r2", [F, F], bf16)
    fA = din("fA", [F, F], bf16)               # W_q W_k^T (lhsT layout A^T)
    fw_vo = din("fw_vo", [F, F], bf16)
    ucol = din("ucol", [F, 1], bf16)
    vcol = din("vcol", [F, 1], bf16)
    bc_amp = din("bc_amp", [F, 1])
    bc_ph = din("bc_ph", [F, 1])
    bc_gate = din("bc_gate", [F, 1])
    bc_r1 = din("bc_r1", [F, 1])
    bc_r2 = din("bc_r2", [F, 1])
    bvo = din("bvo", [F, 1])
    bc_out = din("bc_out", [F, 1])
    c0t = din("c0t", [1, 1])
    identd = din("identd", [128, 128])
    ident2 = din("ident2", [128, 128])
    onesr = din("onesr", [1, 1], bf16)
    regw = din("regw", [H, 2], bf16)
    regb = din("regb", [1, 2], bf16)

    outd = nc.dram_tensor("out", [BS, 2], f32, kind="ExternalOutput").ap()
    dbg = nc.dram_tensor("dbg", [F, TS * BS], bf16, kind="ExternalOutput").ap() \
        if DEBUG_GBUF else None
    dbg2 = nc.dram_tensor("dbg2", [DB + 1, TS * BS], bf16,
                          kind="ExternalOutput").ap() if DEBUG_GBUF else None
    dbg3 = nc.dram_tensor("dbg3", [128, ZW], bf16,
                          kind="ExternalOutput").ap() if DEBUG_GBUF else None

    SC = float(F) ** -0.5
    NPT = TS // TB             # prepass tiles
    NFC = (TS + FT - 1) // FT  # fusion chunks
    PROP = min(8, NPT)
    PROF = min(3, NFC)

    # gate-block column helpers: block (l, gi) = cols (l*4+gi)*16
    def blk(l_, gi):
        o = (l_ * 4 + gi) * 16
        return o, o + 16

    with tile.TileContext(nc) as tc:
        with tc.tile_pool(name="const", bufs=1) as cp:
            def load(name, shape, src, dt=bf16):
                t_ = cp.tile(shape, dt, tag=name)
                nc.sync.dma_start(t_[:], src)
                return t_

            s_w0xa = load("s_w0xa", [DA, NG], w0xa[:])
            s_w0xbb = load("s_w0xbb", [DB + 1, NG], w0xbb[:])
            s_w0h = load("s_w0h", [H, NG], w0h[:])
            s_gw0 = load("s_gw0", [F, NG], gw0[:])
            s_w1x = load("s_w1x", [H, NG], w1x[:])
            s_w1h = load("s_w1h", [H, NG], w1h[:])
            s_gw1 = load("s_gw1", [F, NG], gw1[:])
            s_w2x = load("s_w2x", [H, NG], w2x[:])
            s_w2h = load("s_w2h", [H, NG], w2h[:])
            s_gw2 = load("s_gw2", [F, NG], gw2[:])
            s_bstk = load("s_bstk", [8, 128], bstk[:])
            s_bind = load("s_bind", [8, ZW], bind[:])
            def load_r(name, shape, src):
                t_ = cp.tile(shape, f32, tag=name)
                nc.gpsimd.dma_start(_R(t_), src)
                return t_

            s_fw_amp = load_r("s_fw_amp", [1, F], fw_amp[:])
            s_fw_ph = load_r("s_fw_ph", [1, F], fw_ph[:])
            s_fw_r1 = load_r("s_fw_r1", [2, F], fw_r1[:])
            s_fwg_ph = load("s_fwg_ph", [F, F], fwg_ph[:])
            s_fwg_am = load("s_fwg_am", [F, F], fwg_am[:])
            s_fw_r2 = load("s_fw_r2", [F, F], fw_r2[:])
            s_fA = load("s_fA", [F, F], fA[:])
            s_fw_vo = load("s_fw_vo", [F, F], fw_vo[:])
            s_u = load("s_u", [F, 1], ucol[:])
            s_v = load("s_v", [F, 1], vcol[:])
            s_bc_amp = load("s_bc_amp", [F, 1], bc_amp[:], f32)
            s_bc_ph = load("s_bc_ph", [F, 1], bc_ph[:], f32)
            s_bc_gate = load("s_bc_gate", [F, 1], bc_gate[:], f32)
            s_bc_r1 = load("s_bc_r1", [F, 1], bc_r1[:], f32)
            s_bc_r2 = load("s_bc_r2", [F, 1], bc_r2[:], f32)
            s_bvo = load("s_bvo", [F, 1], bvo[:], f32)
            s_bc_out = load("s_bc_out", [F, 1], bc_out[:], f32)
            s_c0 = load("s_c0", [1, 1], c0t[:], f32)
            s_ident = load("s_ident", [128, 128], identd[:], f32)
            s_ident2 = load_r("s_ident2", [128, 128], ident2[:])
            s_regw = load("s_regw", [H, 2], regw[:])
            s_regb = load("s_regb", [1, 2], regb[:])

            s_ones1 = cp.tile([128, 1], bf16, tag="s_ones1")
            nc.vector.memset(s_ones1[:], 1.0)
            s_ones32 = cp.tile([1, 32], bf16, tag="s_ones32")
            nc.vector.memset(s_ones32[:], 1.0)
            zero48 = cp.tile([128, 48], f32, tag="zero48")
            nc.vector.memset(zero48[:], 0.0)

            # SBUF-resident transposed |x| (bf16) and gamma_sh (bf16)
            xa_s = cp.tile([DA, TS * BS], bf16, tag="xa_s")
            xb_s = cp.tile([DB + 1, TS * BS], bf16, tag="xb_s")
            # ones row for L0 bias: DMA broadcast (engine ops need 32-aligned
            # partition starts; DMA does not)
            nc.sync.dma_start(
                xb_s[DB:DB + 1, :].rearrange("p (a b) -> p a b", a=TS * BS),
                bass.AP(tensor=onesr.tensor, offset=0,
                        ap=[[0, 1], [0, TS * BS], [0, 1]]))
            gbuf = cp.tile([F, TS * BS], bf16, tag="gbuf")
            nc.vector.memset(gbuf[:, 0:BS], 0.0)        # gamma_sh[0] = 0

            W_per = {
                0: (s_gw0, None, s_w0h),
                1: (s_gw1, s_w1x, s_w1h),
                2: (s_gw2, s_w2x, s_w2h),
            }

            hr3 = hrrp.rearrange("(b tt) d -> b tt d", b=BS)

            with (
                tc.tile_pool(name="work", bufs=2) as wp,
                tc.tile_pool(name="zps", bufs=2, space="PSUM") as zp,
                tc.tile_pool(name="fps", bufs=2, space="PSUM") as fp,
                tc.tile_pool(name="tps", bufs=1, space="PSUM") as tp,
            ):
                # ---------- prepass (2 stages per tile) ----------
                pre_state = {}

                def prepass_a(k):
                    t0 = k * TB
                    raw = wp.tile([128, D], f32, tag="raw", name=f"raw{k}", bufs=3)
                    nc.sync.dma_start(
                        raw[:], hr3[:, t0:t0 + TB, :].rearrange("b tt d -> tt b d"))
                    ab = wp.tile([128, D], f32, tag="ab", name=f"ab{k}", bufs=3)
                    nc.vector.tensor_scalar(
                        ab[:].bitcast(u32), raw[:].bitcast(u32),
                        0x7FFFFFFF, None, ALU.bitwise_and)
                    pre_state[k] = ab

                def prepass_b(k):
                    ab = pre_state.pop(k)
                    pt1 = tp.tile([128, 128], f32, tag="ptx", name=f"pt1_{k}")
                    nc.tensor.transpose(pt1[0:DA, :], ab[:, 0:DA], s_ident[:, :])
                    pt2 = tp.tile([128, 128], f32, tag="ptx", name=f"pt2_{k}")
                    nc.tensor.transpose(pt2[0:DB, :], ab[:, DA:D], s_ident[:, :])
                    c0_ = k * 128
                    nc.vector.tensor_copy(xa_s[:, c0_:c0_ + 128], pt1[0:DA, :])
                    nc.vector.tensor_copy(xb_s[0:DB, c0_:c0_ + 128], pt2[0:DB, :])

                # ---------- fusion (16 stages per chunk) ----------
                fu_state = {}

                def fusion_stage(j, st):
                    tj = j * FT
                    N = FT * BS
                    fs = fu_state.setdefault(j, {})
                    if st == 0:
                        a_row = wp.tile([1, N], f32, tag="a_row", name=f"a_row{j}")
                        nc.sync.dma_start(
                            a_row[:].bitcast(f32r).rearrange(
                                "p (tt b) -> p tt b", tt=FT),
                            bass.AP(tensor=ac.tensor, offset=tj,
                                    ap=[[0, 1], [1, FT], [TS, BS]]))
                        p_row = wp.tile([1, N], f32, tag="p_row", name=f"p_row{j}")
                        nc.sync.dma_start(
                            p_row[:].bitcast(f32r).rearrange(
                                "p (tt b) -> p tt b", tt=FT),
                            bass.AP(tensor=pc.tensor, offset=tj,
                                    ap=[[0, 1], [1, FT], [TS, BS]]))
                        rl2 = wp.tile([2, N], f32, tag="rl2", name=f"rl2{j}")
                        for c_ in range(2):
                            nc.sync.dma_start(
                                rl2[:].bitcast(f32r)[c_:c_ + 1, :].rearrange(
                                    "p (tt b) -> p tt b", tt=FT),
                                bass.AP(tensor=rldel.tensor, offset=tj * 2 + c_,
                                        ap=[[0, 1], [2, FT], [2 * TS, BS]]))
                        fs.update(a_row=a_row, p_row=p_row, rl2=rl2)
                    elif st == 1:
                        pA = fp.tile([F, N], f32, tag="fps", name=f"pA{j}")
                        nc.tensor.matmul(pA[:], _R(s_fw_amp),
                                         fs["a_row"][:].bitcast(f32r),
                                         start=True, stop=True)
                        pB = fp.tile([F, N], f32, tag="fps", name=f"pB{j}")
                        nc.tensor.matmul(pB[:], _R(s_fw_ph),
                                         fs["p_row"][:].bitcast(f32r),
                                         start=True, stop=True)
                        fs.update(pA=pA, pB=pB)
                    elif st == 2:
                        ampT = wp.tile([F, N], bf16, tag="ampT", name=f"ampT{j}")
                        nc.scalar.activation(ampT[:], fs["pA"][:], ACTF.Tanh,
                                             bias=s_bc_amp[:])
                        fs["ampT"] = ampT
                    elif st == 3:
                        phT = wp.tile([F, N], bf16, tag="phT", name=f"phT{j}")
                        nc.scalar.activation(phT[:], fs["pB"][:], ACTF.Tanh,
                                             bias=s_bc_ph[:])
                        fs["phT"] = phT
                    elif st == 4:
                        pC = fp.tile([F, N], f32, tag="fps", name=f"pC{j}")
                        nc.tensor.matmul(pC[:], s_fwg_ph[:], fs["phT"][:],
                                         start=True, stop=False)
                        nc.tensor.matmul(pC[:], s_fwg_am[:], fs["ampT"][:],
                                         start=False, stop=True)
                        fs["pC"] = pC
                    elif st == 5:
                        betaT = wp.tile([F, N], bf16, tag="betaT", name=f"betaT{j}")
                        nc.scalar.activation(betaT[:], fs["pC"][:], ACTF.Sigmoid,
                                             bias=s_bc_gate[:])
                        fs["betaT"] = betaT
                    elif st == 6:
                        dT = wp.tile([F, N], bf16, tag="dT", name=f"dT{j}")
                        nc.vector.tensor_tensor(dT[:], fs["phT"][:], fs["ampT"][:],
                                                ALU.subtract)
                        mT = wp.tile([F, N], bf16, tag="mT", name=f"mT{j}")
                        nc.vector.tensor_tensor(mT[:], fs["betaT"][:], dT[:],
                                                ALU.mult)
                        corrT = wp.tile([F, N], bf16, tag="corrT", name=f"corrT{j}")
                        nc.vector.tensor_tensor(corrT[:], mT[:], fs["ampT"][:],
                                                ALU.add)
                        fs["corrT"] = corrT
                    elif st == 7:
                        pR1 = fp.tile([F, N], f32, tag="fps", name=f"pR1{j}")
                        nc.tensor.matmul(pR1[:], _R(s_fw_r1),
                                         fs["rl2"][:].bitcast(f32r),
                                         start=True, stop=True)
                        fs["pR1"] = pR1
                    elif st == 8:
                        rl1T = wp.tile([F, N], bf16, tag="rl1T", name=f"rl1T{j}")
                        nc.scalar.activation(rl1T[:], fs["pR1"][:], ACTF.Tanh,
                                             bias=s_bc_r1[:])
                        fs["rl1T"] = rl1T
                    elif st == 9:
                        pR2 = fp.tile([F, N], f32, tag="fps", name=f"pR2{j}")
                        nc.tensor.matmul(pR2[:], s_fw_r2[:], fs["rl1T"][:],
                                         start=True, stop=True)
                        fs["pR2"] = pR2
                    elif st == 10:
                        rlT = wp.tile([F, N], bf16, tag="rlT", name=f"rlT{j}")
                        nc.scalar.activation(rlT[:], fs["pR2"][:], ACTF.Tanh,
                                             bias=s_bc_r2[:])
                        fs["rlT"] = rlT
                    elif st == 11:
                        pAr = fp.tile([F, N], f32, tag="fps", name=f"pAr{j}")
                        nc.tensor.matmul(pAr[:], s_fA[:], fs["rlT"][:],
                                         start=True, stop=True)
                        wT = wp.tile([F, N], bf16, tag="wT", name=f"wT{j}")
                        nc.vector.tensor_tensor(wT[:], fs["corrT"][:], pAr[:],
                                                ALU.mult)
                        fs["wT"] = wT
                    elif st == 12:
                        pS = fp.tile([F, N], f32, tag="fps", name=f"pS{j}")
                        nc.tensor.matmul(pS[0:1, :], s_ones1[:], fs["wT"][:],
                                         start=True, stop=False)
                        nc.tensor.matmul(pS[0:1, :], s_u[:], fs["corrT"][:],
                                         start=False, stop=False)
                        nc.tensor.matmul(pS[0:1, :], s_v[:], fs["rlT"][:],
                                         start=False, stop=True)
                        fs["pS"] = pS
                    elif st == 13:
                        attnT = wp.tile([1, N], bf16, tag="attnT", name=f"attnT{j}")
                        nc.scalar.activation(attnT[:], fs["pS"][0:1, :],
                                             ACTF.Sigmoid, bias=s_c0[:], scale=SC)
                        fs["attnT"] = attnT
                    elif st == 14:
                        abc = wp.tile([F, N], bf16, tag="abc", name=f"abc{j}")
                        nc.gpsimd.partition_broadcast(abc[:], fs["attnT"][:])
                        pG = fp.tile([F, N], f32, tag="fps", name=f"pG{j}")
                        nc.tensor.matmul(pG[:], s_fw_vo[:], fs["rlT"][:],
                                         start=True, stop=True)
                        fs.update(abc=abc, pG=pG)
                    elif st == 15:
                        tmpT = wp.tile([F, N], bf16, tag="tmpT", name=f"tmpT{j}")
                        nc.vector.scalar_tensor_tensor(
                            tmpT[:], fs["pG"][:], s_bvo[:], fs["abc"][:],
                            ALU.add, ALU.mult)
                        nrow = min(FT, TS - 1 - tj)
                        nc.vector.tensor_scalar(
                            gbuf[:, (tj + 1) * BS:(tj + 1 + nrow) * BS],
                            tmpT[:, 0:nrow * BS], s_bc_out[:], None, ALU.add)
                        fu_state.pop(j)

                # ---------- recurrence ----------
                GWt, WXt, WHt = W_per[0][0], None, None  # noqa

                def new_ztile(tag, name):
                    """z tile + its single start=True chain head: zeroes all
                    240 cols, deposits L1/L2 biases (start=True on any region
                    resets the whole accumulation bank, so exactly one)."""
                    z = zp.tile([128, ZW], f32, tag=tag, name=name)
                    nc.tensor.matmul(z[:], s_bstk[:], s_bind[:],
                                     start=True, stop=False)
                    return z

                def emit_gates(z, g, s, hcur):
                    """all matmuls for (group g, step s) into z[:, 0:192]."""
                    t0 = min(s, TS - 1)
                    t1 = min(max(s - 1, 0), TS - 1)
                    t2 = min(max(s - 2, 0), TS - 1)
                    cA = t0 * BS + g * G
                    # L0 x-chunks
                    for gi in range(4):
                        a, b_ = blk(0, gi)
                        nc.tensor.matmul(
                            z[:, a:b_], s_w0xa[:, gi * H:(gi + 1) * H],
                            xa_s[:, cA:cA + G], start=False, stop=False)
                        nc.tensor.matmul(
                            z[:, a:b_], s_w0xbb[:, gi * H:(gi + 1) * H],
                            xb_s[:, cA:cA + G], start=False, stop=False)
                    # gammas
                    for l_, tl in ((0, t0), (1, t1), (2, t2)):
                        gw = W_per[l_][0]
                        cG = tl * BS + g * G
                        for gi in range(4):
                            a, b_ = blk(l_, gi)
                            nc.tensor.matmul(
                                z[:, a:b_], gw[:, gi * H:(gi + 1) * H],
                                gbuf[:, cG:cG + G], start=False, stop=False)
                    # h matmuls (critical path: emitted last)
                    for gi in range(4):
                        a, b_ = blk(0, gi)
                        nc.tensor.matmul(
                            z[:, a:b_], s_w0h[:, gi * H:(gi + 1) * H],
                            hcur[:, 0:G], start=False, stop=True)
                    for l_ in (1, 2):
                        wx, wh = W_per[l_][1], W_per[l_][2]
                        hin = hcur[:, (l_ - 1) * G:l_ * G]
                        hown = hcur[:, l_ * G:(l_ + 1) * G]
                        for gi in range(4):
                            a, b_ = blk(l_, gi)
                            nc.tensor.matmul(z[:, a:b_],
                                             wx[:, gi * H:(gi + 1) * H], hin,
                                             start=False, stop=False)
                            nc.tensor.matmul(z[:, a:b_],
                                             wh[:, gi * H:(gi + 1) * H], hown,
                                             start=False, stop=True)

                def s3(t_, gi):
                    """[128, 3, 16] AP over gate gi of all 3 layers."""
                    return t_[:, 0:C2O].rearrange(
                        "p (l g x) -> p l g x", l=3, g=4)[:, :, gi, :]

                def r3(t_):
                    return t_[:].rearrange("p (l x) -> p l x", l=3)

                def emit_sigma(z, g, s):
                    S = wp.tile([128, ZW], bf16, tag=f"S{g}", name=f"S{g}_{s}")
                    nc.scalar.activation(S[:], z[:], ACTF.Sigmoid)
                    if s == 0:
                        nc.vector.memset(S[:, 64:C2O], 0.0)
                    elif s == 1:
                        nc.vector.memset(S[:, 128:C2O], 0.0)
                    return S

                def emit_h(S_c2src, S_own, g, s):
                    """h(g) = sigma_o(own) * (2*sigma(2c)-1); c2 cols from S_c2src."""
                    thm = wp.tile([128, 48], bf16, tag="thm", name=f"thm{g}_{s}",
                                  bufs=4)
                    nc.vector.tensor_scalar(thm[:], S_c2src[:, C2O:ZW], 2.0, 1.0,
                                            ALU.mult, ALU.subtract)
                    h = wp.tile([128, 48], bf16, tag=f"h{g}", name=f"h{g}_{s}",
                                bufs=3)
                    nc.vector.tensor_tensor(r3(h), s3(S_own, 2), r3(thm), ALU.mult)
                    return h

                def emit_cpath(S, csrc, z_dst, g, s):
                    """c(g,s) = sigma_i*c_hat + sigma_f*c_old (SBUF f32);
                    2c -> z_dst c2 psum region via PE 2I matmul."""
                    ct = wp.tile([128, 48], bf16, tag="ct", name=f"ct{g}_{s}",
                                 bufs=4)
                    nc.vector.tensor_scalar(r3(ct), s3(S, 3), 2.0, 1.0,
                                            ALU.mult, ALU.subtract)
                    m1 = wp.tile([128, 48], bf16, tag="m1", name=f"m1{g}_{s}",
                                 bufs=4)
                    nc.vector.tensor_tensor(r3(m1), s3(S, 0), r3(ct), ALU.mult)
                    m2 = wp.tile([128, 48], f32, tag="m2", name=f"m2{g}_{s}",
                                 bufs=4)
                    nc.gpsimd.tensor_tensor(r3(m2), s3(S, 1), csrc, ALU.mult)
                    cnew = wp.tile([128, 48], f32, tag=f"c{g}",
                                   name=f"c{g}_{s}", bufs=2)
                    nc.gpsimd.tensor_tensor(cnew[:].bitcast(f32r), m1[:], m2[:],
                                            ALU.add)
                    nc.tensor.matmul(z_dst[:, C2O:ZW], _R(s_ident2),
                                     cnew[:].bitcast(f32r), start=False, stop=True)
                    return r3(cnew)

                # initial state
                hA = cp.tile([128, 48], bf16, tag="hA0")
                nc.vector.memset(hA[:], 0.0)
                hB = cp.tile([128, 48], bf16, tag="hB0")
                nc.vector.memset(hB[:], 0.0)
                SB_prev = cp.tile([128, ZW], bf16, tag="SB_init")
                nc.vector.memset(SB_prev[:], 0.0)
                c2A_src = zero48[:].rearrange("p (l x) -> p l x", l=3)
                c2B_src = c2A_src

                pA_cur = new_ztile("zA", "zA_0")

                for k in range(PROP):
                    prepass_a(k)
                    prepass_b(k)
                for j in range(PROF):
                    for st in range(FT):
                        fusion_stage(j, st)

                for s in range(STOT):
                    if s % TB == 0:
                        k = s // TB + PROP
                        if k < NPT:
                            prepass_a(k)
                    elif s % TB == 2:
                        k = s // TB + PROP
                        if k < NPT:
                            prepass_b(k)
                    jf = s // FT + PROF
                    if jf < NFC:
                        fusion_stage(jf, s % FT)

                    # --- group A step s ---
                    emit_gates(pA_cur, 0, s, hA)
                    SA = emit_sigma(pA_cur, 0, s)
                    if DEBUG_GBUF and s == DEBUG_S:
                        nc.sync.dma_start(dbg3[:], SA[:])
                    # h_B(s-1) from SA's c2 region + SB_prev's o-gates
                    hB = emit_h(SA, SB_prev, 1, s)
                    # --- group B PE ---
                    pB_cur = new_ztile("zB", f"zB_{s}")
                    emit_gates(pB_cur, 1, s, hB)
                    # A c-path -> pB c2 region
                    c2A_src = emit_cpath(SA, c2A_src, pB_cur, 0, s)
                    SB = emit_sigma(pB_cur, 1, s)
                    # h_A(s) from SB c2 region + SA o-gates
                    hA = emit_h(SB, SA, 0, s)
                    # B c-path -> next pA tile
                    pA_next = new_ztile("zA", f"zA_{s + 1}")
                    c2B_src = emit_cpath(SB, c2B_src, pA_next, 1, s)
                    pA_cur = pA_next
                    SB_prev = SB

                # ---------- output: h2 = sigma_o2 * tanh(c2/2) ----------
                lastA_c2 = pA_cur           # 2c_B(STOT-1) in [C2O+32:ZW] l=2 slice
                lastB_c2 = pB_cur           # 2c_A(STOT-1)
                thA = wp.tile([128, 16], bf16, tag="thA_f")
                nc.scalar.activation(thA[:], lastB_c2[:, C2O + 32:ZW], ACTF.Tanh,
                                     scale=0.5)
                thB = wp.tile([128, 16], bf16, tag="thB_f")
                nc.scalar.activation(thB[:], lastA_c2[:, C2O + 32:ZW], ACTF.Tanh,
                                     scale=0.5)
                h2A = wp.tile([128, 16], bf16, tag="h2A")
                nc.vector.tensor_tensor(h2A[:], SA[:, 160:176], thA[:], ALU.mult)
                h2B = wp.tile([128, 16], bf16, tag="h2B")
                nc.vector.tensor_tensor(h2B[:], SB[:, 160:176], thB[:], ALU.mult)

                for gi_, h2g in ((0, h2A), (1, h2B)):
                    pog = tp.tile([128, 128], f32, tag="ptx", name=f"po{gi_}")
                    nc.tensor.matmul(pog[0:16, 0:2], h2g[:], s_regw[:],
                                     start=True, stop=False)
                    nc.tensor.matmul(pog[0:16, 0:2], s_ones32[:, 0:16], s_regb[:],
                                     start=False, stop=True)
                    og = wp.tile([16, 2], f32, tag="outs", name=f"outs{gi_}")
                    nc.scalar.copy(og[:], pog[0:16, 0:2])
                    nc.sync.dma_start(outd[gi_ * 16:(gi_ + 1) * 16, :], og[:])
                if DEBUG_GBUF:
                    nc.sync.dma_start(dbg[:], gbuf[:])
                    nc.sync.dma_start(dbg2[:], xb_s[:])

    nc.compile()
    return nc


def prep_inputs(inputs, t_steps=T):
    TS = t_steps

    def g(k):
        return np.asarray(inputs[k], dtype=np.float32)

    def b(x):
        return np.ascontiguousarray(x.astype(BF))

    perm = np.concatenate([np.arange(0, H), np.arange(H, 2 * H),
                           np.arange(3 * H, 4 * H), np.arange(2 * H, 3 * H)])

    base_w0 = g("base_w0")[:, perm]
    w0h = base_w0[:H].copy()
    w0x = base_w0[H:]
    w0xa = w0x[:DA].copy()
    w0xb = w0x[DA:].copy()
    b0 = g("base_b0")[perm]
    bw12 = g("base_w12")
    w1 = bw12[0][:, perm]
    w2 = bw12[1][:, perm]
    w1h, w1x = w1[:H].copy(), w1[H:].copy()
    w2h, w2x = w2[:H].copy(), w2[H:].copy()
    b12 = g("base_b12")
    b1, b2 = b12[0][perm].copy(), b12[1][perm].copy()

    def gwstack(gw):
        return np.concatenate([gw[0], -LAM * gw[1], gw[3], gw[2]], axis=1)

    gw0 = gwstack(g("gam_w0"))
    gw12 = g("gam_w12")
    gw1, gw2 = gwstack(gw12[0]), gwstack(gw12[1])

    w0xbb = np.concatenate([w0xb, b0[None, :]], axis=0)
    for arr in (w0h, w0xa, w0xbb, gw0, w1h, w1x, gw1, w2h, w2x, gw2, b1, b2):
        arr[..., 3 * H:4 * H] *= 2.0

    # L1/L2 bias stack [8, 128]: rows = (L1 i,f,o,c, L2 i,f,o,c) bias vectors
    bstk = np.stack([b1[i * H:(i + 1) * H] for i in range(4)]
                    + [b2[i * H:(i + 1) * H] for i in range(4)])
    bind = np.zeros((8, 240), np.float32)
    for k in range(8):
        bind[k, 64 + k * 16:64 + (k + 1) * 16] = 1.0

    f_q_w, f_k_w = g("f_q_w"), g("f_k_w")
    f_q_b, f_k_b = g("f_q_b"), g("f_k_b")
    A = f_q_w @ f_k_w.T
    u = f_q_w @ f_k_b
    v = f_k_w @ f_q_b
    c0 = float(f_q_b @ f_k_b)
    fw_vo = g("f_v_w") @ g("f_out_w")
    bvo = g("f_v_b") @ g("f_out_w")

    consts = {
        "w0xa": b(w0xa), "w0xbb": b(w0xbb), "w0h": b(w0h), "gw0": b(gw0),
        "w1x": b(w1x), "w1h": b(w1h), "gw1": b(gw1),
        "w2x": b(w2x), "w2h": b(w2h), "gw2": b(gw2),
        "bstk": b(bstk), "bind": b(bind),
        "fw_amp": g("f_amp_w"), "fw_ph": g("f_ph_w"), "fw_r1": g("f_rlos_w1"),
        "fwg_ph": b(g("f_gate_w")[0:F]), "fwg_am": b(g("f_gate_w")[F:2 * F]),
        "fw_r2": b(g("f_rlos_w2")),
        "fA": b(A.T), "fw_vo": b(fw_vo),
        "ucol": b(u[:, None]), "vcol": b(v[:, None]),
        "bc_amp": g("f_amp_b")[:, None], "bc_ph": g("f_ph_b")[:, None],
        "bc_gate": g("f_gate_b")[:, None], "bc_r1": g("f_rlos_b1")[:, None],
        "bc_r2": g("f_rlos_b2")[:, None],
        "bvo": bvo[:, None].astype(np.float32),
        "bc_out": g("f_out_b")[:, None],
        "c0t": np.full((1, 1), c0, np.float32),
        "identd": np.eye(128, dtype=np.float32),
        "ident2": 2.0 * np.eye(128, dtype=np.float32),
        "onesr": np.ones((1, 1), BF),
        "regw": b(g("reg_w")), "regb": b(g("reg_b")[None, :]),
    }
    consts = {k: np.ascontiguousarray(v) for k, v in consts.items()}

    hrrp = g("hrrp")[:, :TS, :]
    ac = g("amplitude_corr")[:, :TS]
    pc_ = g("phase_corr")[:, :TS]
    rldel = g("rlos_delta")[:, :TS, :]

    in_maps = []
    for c in range(NCORES):
        sl = slice(c * BS, (c + 1) * BS)
        m = dict(consts)
        m["hrrp"] = np.ascontiguousarray(hrrp[sl].reshape(BS * TS, D))
        m["ac"] = np.ascontiguousarray(ac[sl])
        m["pc"] = np.ascontiguousarray(pc_[sl])
        m["rldel"] = np.ascontiguousarray(rldel[sl])
        in_maps.append(m)
    return in_maps


_NC_CACHE = {}


def _get_nc(t_steps=T):
    if t_steps not in _NC_CACHE:
        _NC_CACHE[t_steps] = build_nc(t_steps)
    return _NC_CACHE[t_steps]


def run(inputs, t_steps=T, **kwargs):
    nc = _get_nc(t_steps)
    in_maps = prep_inputs(inputs, t_steps)
    res = run_bass_kernel_spmd(nc, in_maps, core_ids=list(range(NCORES)), **kwargs)
    out = np.concatenate([res.results[c]["out"] for c in range(NCORES)], axis=0)
    return out, res


def kernel(**inputs) -> np.ndarray:
    out, _ = run(inputs)
    return out.astype(np.float32)


# revision 13
# speedup vs baseline: 1.6699x; 1.6290x over previous
"""Trainium2 Bass kernel for nn_CGLSTM (TwoStageFusion + 3-layer gamma-modulated LSTM).

v2: transposed (feature-major) bf16 recurrence.

Sharding: pure data parallel over batch B=256 across 8 NeuronCores (32/core),
split into 2 interleaved groups of 16 to pipeline the serial recurrence.

Per step s and group g, one PSUM tile [128, 240] holds z^T for all 3 wavefront
layers (12 gate-blocks x 16 batch, gate cols pre-scaled so tanh folds into
sigmoid) plus a 48-col region where the *other* group's 2*c state was written
by the vector engines; ONE sigmoid instruction then yields all gates and
tanh(c_other) at once.  Gate matmuls are bf16 with the weights stationary
(out free size 16), x^T and gamma^T live SBUF-resident, so the inner loop
does no DMA and no transposes.
"""

import sys

sys.path.insert(0, "/opt/trn_rl_repo")

import numpy as np  # noqa: E402
import ml_dtypes  # noqa: E402

import concourse.bass as bass  # noqa: E402, F401
import concourse.tile as tile  # noqa: E402
from concourse import bacc, mybir  # noqa: E402
from concourse.bass_utils import run_bass_kernel_spmd  # noqa: E402

f32 = mybir.dt.float32
f32r = mybir.dt.float32r
bf16 = mybir.dt.bfloat16
u32 = mybir.dt.uint32
ACTF = mybir.ActivationFunctionType
ALU = mybir.AluOpType
BF = ml_dtypes.bfloat16

B, T, D, H, F = 256, 512, 200, 128, 128
LAM = 0.5
NCORES = 8
BS = B // NCORES   # 32
G = BS // 2        # 16 per group
DA, DB = 128, D - 128
NG = 4 * H
FT = 16            # fusion chunk t-size (= #stages)
TB = 4             # prepass t's per [128,200] tile
DEBUG_GBUF = False
DEBUG_S = 0
C2O = 192          # c2 region offset in the z psum tile
ZW = C2O + 48      # 240


def _R(t):
    return t[:].bitcast(f32r)


def build_nc(t_steps=T):
    TS = t_steps
    STOT = TS + 2
    nc = bacc.Bacc("TRN2", target_bir_lowering=False, debug=False, num_devices=NCORES)

    def din(name, shape, dt=f32):
        return nc.dram_tensor(name, shape, dt, kind="ExternalInput").ap()

    hrrp = din("hrrp", [BS * TS, D])
    ac = nc.dram_tensor("ac", [BS, TS], f32r, kind="ExternalInput").ap()
    pc = nc.dram_tensor("pc", [BS, TS], f32r, kind="ExternalInput").ap()
    rldel = nc.dram_tensor("rldel", [BS, TS, 2], f32r, kind="ExternalInput").ap()
    # recurrence weights (bf16, gate order [i f o c], c-cols pre-scaled x2)
    w0xa = din("w0xa", [DA, NG], bf16)
    w0xbb = din("w0xbb", [DB + 1, NG], bf16)   # row DB = L0 bias
    w0h = din("w0h", [H, NG], bf16)
    gw0 = din("gw0", [F, NG], bf16)
    w1x = din("w1x", [H, NG], bf16)
    w1h = din("w1h", [H, NG], bf16)
    gw1 = din("gw1", [F, NG], bf16)
    w2x = din("w2x", [H, NG], bf16)
    w2h = din("w2h", [H, NG], bf16)
    gw2 = din("gw2", [F, NG], bf16)
    bstk = din("bstk", [8, 128], bf16)         # L1/L2 biases stacked
    bind = din("bind", [8, C2O], bf16)         # bias block indicator (zero-padded)
    # fusion weights
    fw_amp = din("fw_amp", [1, F])
    fw_ph = din("fw_ph", [1, F])
    fw_r1 = din("fw_r1", [2, F])
    fwg_ph = din("fwg_ph", [F, F], bf16)
    fwg_am = din("fwg_am", [F, F], bf16)
    fw_r2 = din("fw_r2", [F, F], bf16)
    fA = din("fA", [F, F], bf16)               # W_q W_k^T (lhsT layout A^T)
    fw_vo = din("fw_vo", [F, F], bf16)
    ucol = din("ucol", [F, 1], bf16)
    vcol = din("vcol", [F, 1], bf16)
    bc_amp = din("bc_amp", [F, 1])
    bc_ph = din("bc_ph", [F, 1])
    bc_gate = din("bc_gate", [F, 1])
    bc_r1 = din("bc_r1", [F, 1])
    bc_r2 = din("bc_r2", [F, 1])
    bvo = din("bvo", [F, 1])
    bc_out = din("bc_out", [F, 1])
    c0t = din("c0t", [1, 1])
    identd = din("identd", [128, 128])
    ident2 = din("ident2", [128, 128])
    onesr = din("onesr", [1, 1], bf16)
    regw = din("regw", [H, 2], bf16)
    regb = din("regb", [1, 2], bf16)

    outd = nc.dram_tensor("out", [BS, 2], f32, kind="ExternalOutput").ap()
    dbg = nc.dram_tensor("dbg", [F, TS * BS], bf16, kind="ExternalOutput").ap() \
        if DEBUG_GBUF else None
    dbg2 = nc.dram_tensor("dbg2", [DB + 1, TS * BS], bf16,
                          kind="ExternalOutput").ap() if DEBUG_GBUF else None
    dbg3 = nc.dram_tensor("dbg3", [128, C2O], bf16,
                          kind="ExternalOutput").ap() if DEBUG_GBUF else None

    SC = float(F) ** -0.5
    NPT = TS // TB             # prepass tiles
    NFC = (TS + FT - 1) // FT  # fusion chunks
    PROP = min(8, NPT)
    PROF = min(3, NFC)

    # gate-block column helpers: block (l, gi) = cols (l*4+gi)*16
    def blk(l_, gi):
        o = (l_ * 4 + gi) * 16
        return o, o + 16

    with tile.TileContext(nc) as tc:
        with tc.tile_pool(name="const", bufs=1) as cp:
            def load(name, shape, src, dt=bf16):
                t_ = cp.tile(shape, dt, tag=name)
                nc.sync.dma_start(t_[:], src)
                return t_

            s_w0xa = load("s_w0xa", [DA, NG], w0xa[:])
            s_w0xbb = load("s_w0xbb", [DB + 1, NG], w0xbb[:])
            s_w0h = load("s_w0h", [H, NG], w0h[:])
            s_gw0 = load("s_gw0", [F, NG], gw0[:])
            s_w1x = load("s_w1x", [H, NG], w1x[:])
            s_w1h = load("s_w1h", [H, NG], w1h[:])
            s_gw1 = load("s_gw1", [F, NG], gw1[:])
            s_w2x = load("s_w2x", [H, NG], w2x[:])
            s_w2h = load("s_w2h", [H, NG], w2h[:])
            s_gw2 = load("s_gw2", [F, NG], gw2[:])
            s_bstk = load("s_bstk", [8, 128], bstk[:])
            s_bind = load("s_bind", [8, C2O], bind[:])
            def load_r(name, shape, src):
                t_ = cp.tile(shape, f32, tag=name)
                nc.gpsimd.dma_start(_R(t_), src)
                return t_

            s_fw_amp = load_r("s_fw_amp", [1, F], fw_amp[:])
            s_fw_ph = load_r("s_fw_ph", [1, F], fw_ph[:])
            s_fw_r1 = load_r("s_fw_r1", [2, F], fw_r1[:])
            s_fwg_ph = load("s_fwg_ph", [F, F], fwg_ph[:])
            s_fwg_am = load("s_fwg_am", [F, F], fwg_am[:])
            s_fw_r2 = load("s_fw_r2", [F, F], fw_r2[:])
            s_fA = load("s_fA", [F, F], fA[:])
            s_fw_vo = load("s_fw_vo", [F, F], fw_vo[:])
            s_u = load("s_u", [F, 1], ucol[:])
            s_v = load("s_v", [F, 1], vcol[:])
            s_bc_amp = load("s_bc_amp", [F, 1], bc_amp[:], f32)
            s_bc_ph = load("s_bc_ph", [F, 1], bc_ph[:], f32)
            s_bc_gate = load("s_bc_gate", [F, 1], bc_gate[:], f32)
            s_bc_r1 = load("s_bc_r1", [F, 1], bc_r1[:], f32)
            s_bc_r2 = load("s_bc_r2", [F, 1], bc_r2[:], f32)
            s_bvo = load("s_bvo", [F, 1], bvo[:], f32)
            s_bc_out = load("s_bc_out", [F, 1], bc_out[:], f32)
            s_c0 = load("s_c0", [1, 1], c0t[:], f32)
            s_ident = load("s_ident", [128, 128], identd[:], f32)
            s_ident2 = load_r("s_ident2", [128, 128], ident2[:])
            s_regw = load("s_regw", [H, 2], regw[:])
            s_regb = load("s_regb", [1, 2], regb[:])

            s_ones1 = cp.tile([128, 1], bf16, tag="s_ones1")
            nc.vector.memset(s_ones1[:], 1.0)
            s_ones32 = cp.tile([1, 32], bf16, tag="s_ones32")
            nc.vector.memset(s_ones32[:], 1.0)
            zero48 = cp.tile([128, 48], f32, tag="zero48")
            nc.vector.memset(zero48[:], 0.0)

            # SBUF-resident transposed |x| (bf16) and gamma_sh (bf16)
            xa_s = cp.tile([DA, TS * BS], bf16, tag="xa_s")
            xb_s = cp.tile([DB + 1, TS * BS], bf16, tag="xb_s")
            # ones row for L0 bias: DMA broadcast (engine ops need 32-aligned
            # partition starts; DMA does not)
            nc.sync.dma_start(
                xb_s[DB:DB + 1, :].rearrange("p (a b) -> p a b", a=TS * BS),
                bass.AP(tensor=onesr.tensor, offset=0,
                        ap=[[0, 1], [0, TS * BS], [0, 1]]))
            gbuf = cp.tile([F, TS * BS], bf16, tag="gbuf")
            nc.vector.memset(gbuf[:, 0:BS], 0.0)        # gamma_sh[0] = 0

            W_per = {
                0: (s_gw0, None, s_w0h),
                1: (s_gw1, s_w1x, s_w1h),
                2: (s_gw2, s_w2x, s_w2h),
            }

            hr3 = hrrp.rearrange("(b tt) d -> b tt d", b=BS)

            with (
                tc.tile_pool(name="work", bufs=2) as wp,
                tc.tile_pool(name="zps", bufs=2, space="PSUM") as zp,
                tc.tile_pool(name="fps", bufs=2, space="PSUM") as fp,
                tc.tile_pool(name="tps", bufs=1, space="PSUM") as tp,
            ):
                # ---------- prepass (2 stages per tile) ----------
                pre_state = {}

                def prepass_a(k):
                    t0 = k * TB
                    raw = wp.tile([128, D], f32, tag="raw", name=f"raw{k}", bufs=3)
                    nc.sync.dma_start(
                        raw[:], hr3[:, t0:t0 + TB, :].rearrange("b tt d -> tt b d"))
                    ab = wp.tile([128, D], f32, tag="ab", name=f"ab{k}", bufs=3)
                    nc.vector.tensor_scalar(
                        ab[:].bitcast(u32), raw[:].bitcast(u32),
                        0x7FFFFFFF, None, ALU.bitwise_and)
                    pre_state[k] = ab

                def prepass_b(k):
                    ab = pre_state.pop(k)
                    pt1 = tp.tile([128, 128], f32, tag="ptx", name=f"pt1_{k}")
                    nc.tensor.transpose(pt1[0:DA, :], ab[:, 0:DA], s_ident[:, :])
                    pt2 = tp.tile([128, 128], f32, tag="ptx", name=f"pt2_{k}")
                    nc.tensor.transpose(pt2[0:DB, :], ab[:, DA:D], s_ident[:, :])
                    c0_ = k * 128
                    nc.vector.tensor_copy(xa_s[:, c0_:c0_ + 128], pt1[0:DA, :])
                    nc.vector.tensor_copy(xb_s[0:DB, c0_:c0_ + 128], pt2[0:DB, :])

                # ---------- fusion (16 stages per chunk) ----------
                fu_state = {}

                def fusion_stage(j, st):
                    tj = j * FT
                    N = FT * BS
                    fs = fu_state.setdefault(j, {})
                    if st == 0:
                        a_row = wp.tile([1, N], f32, tag="a_row", name=f"a_row{j}")
                        nc.sync.dma_start(
                            a_row[:].bitcast(f32r).rearrange(
                                "p (tt b) -> p tt b", tt=FT),
                            bass.AP(tensor=ac.tensor, offset=tj,
                                    ap=[[0, 1], [1, FT], [TS, BS]]))
                        p_row = wp.tile([1, N], f32, tag="p_row", name=f"p_row{j}")
                        nc.sync.dma_start(
                            p_row[:].bitcast(f32r).rearrange(
                                "p (tt b) -> p tt b", tt=FT),
                            bass.AP(tensor=pc.tensor, offset=tj,
                                    ap=[[0, 1], [1, FT], [TS, BS]]))
                        rl2 = wp.tile([2, N], f32, tag="rl2", name=f"rl2{j}")
                        for c_ in range(2):
                            nc.sync.dma_start(
                                rl2[:].bitcast(f32r)[c_:c_ + 1, :].rearrange(
                                    "p (tt b) -> p tt b", tt=FT),
                                bass.AP(tensor=rldel.tensor, offset=tj * 2 + c_,
                                        ap=[[0, 1], [2, FT], [2 * TS, BS]]))
                        fs.update(a_row=a_row, p_row=p_row, rl2=rl2)
                    elif st == 1:
                        pA = fp.tile([F, N], f32, tag="fps", name=f"pA{j}")
                        nc.tensor.matmul(pA[:], _R(s_fw_amp),
                                         fs["a_row"][:].bitcast(f32r),
                                         start=True, stop=True)
                        pB = fp.tile([F, N], f32, tag="fps", name=f"pB{j}")
                        nc.tensor.matmul(pB[:], _R(s_fw_ph),
                                         fs["p_row"][:].bitcast(f32r),
                                         start=True, stop=True)
                        fs.update(pA=pA, pB=pB)
                    elif st == 2:
                        ampT = wp.tile([F, N], bf16, tag="ampT", name=f"ampT{j}")
                        nc.scalar.activation(ampT[:], fs["pA"][:], ACTF.Tanh,
                                             bias=s_bc_amp[:])
                        fs["ampT"] = ampT
                    elif st == 3:
                        phT = wp.tile([F, N], bf16, tag="phT", name=f"phT{j}")
                        nc.scalar.activation(phT[:], fs["pB"][:], ACTF.Tanh,
                                             bias=s_bc_ph[:])
                        fs["phT"] = phT
                    elif st == 4:
                        pC = fp.tile([F, N], f32, tag="fps", name=f"pC{j}")
                        nc.tensor.matmul(pC[:], s_fwg_ph[:], fs["phT"][:],
                                         start=True, stop=False)
                        nc.tensor.matmul(pC[:], s_fwg_am[:], fs["ampT"][:],
                                         start=False, stop=True)
                        fs["pC"] = pC
                    elif st == 5:
                        betaT = wp.tile([F, N], bf16, tag="betaT", name=f"betaT{j}")
                        nc.scalar.activation(betaT[:], fs["pC"][:], ACTF.Sigmoid,
                                             bias=s_bc_gate[:])
                        fs["betaT"] = betaT
                    elif st == 6:
                        dT = wp.tile([F, N], bf16, tag="dT", name=f"dT{j}")
                        nc.vector.tensor_tensor(dT[:], fs["phT"][:], fs["ampT"][:],
                                                ALU.subtract)
                        mT = wp.tile([F, N], bf16, tag="mT", name=f"mT{j}")
                        nc.vector.tensor_tensor(mT[:], fs["betaT"][:], dT[:],
                                                ALU.mult)
                        corrT = wp.tile([F, N], bf16, tag="corrT", name=f"corrT{j}")
                        nc.vector.tensor_tensor(corrT[:], mT[:], fs["ampT"][:],
                                                ALU.add)
                        fs["corrT"] = corrT
                    elif st == 7:
                        pR1 = fp.tile([F, N], f32, tag="fps", name=f"pR1{j}")
                        nc.tensor.matmul(pR1[:], _R(s_fw_r1),
                                         fs["rl2"][:].bitcast(f32r),
                                         start=True, stop=True)
                        fs["pR1"] = pR1
                    elif st == 8:
                        rl1T = wp.tile([F, N], bf16, tag="rl1T", name=f"rl1T{j}")
                        nc.scalar.activation(rl1T[:], fs["pR1"][:], ACTF.Tanh,
                                             bias=s_bc_r1[:])
                        fs["rl1T"] = rl1T
                    elif st == 9:
                        pR2 = fp.tile([F, N], f32, tag="fps", name=f"pR2{j}")
                        nc.tensor.matmul(pR2[:], s_fw_r2[:], fs["rl1T"][:],
                                         start=True, stop=True)
                        fs["pR2"] = pR2
                    elif st == 10:
                        rlT = wp.tile([F, N], bf16, tag="rlT", name=f"rlT{j}")
                        nc.scalar.activation(rlT[:], fs["pR2"][:], ACTF.Tanh,
                                             bias=s_bc_r2[:])
                        fs["rlT"] = rlT
                    elif st == 11:
                        pAr = fp.tile([F, N], f32, tag="fps", name=f"pAr{j}")
                        nc.tensor.matmul(pAr[:], s_fA[:], fs["rlT"][:],
                                         start=True, stop=True)
                        wT = wp.tile([F, N], bf16, tag="wT", name=f"wT{j}")
                        nc.vector.tensor_tensor(wT[:], fs["corrT"][:], pAr[:],
                                                ALU.mult)
                        fs["wT"] = wT
                    elif st == 12:
                        pS = fp.tile([F, N], f32, tag="fps", name=f"pS{j}")
                        nc.tensor.matmul(pS[0:1, :], s_ones1[:], fs["wT"][:],
                                         start=True, stop=False)
                        nc.tensor.matmul(pS[0:1, :], s_u[:], fs["corrT"][:],
                                         start=False, stop=False)
                        nc.tensor.matmul(pS[0:1, :], s_v[:], fs["rlT"][:],
                                         start=False, stop=True)
                        fs["pS"] = pS
                    elif st == 13:
                        attnT = wp.tile([1, N], bf16, tag="attnT", name=f"attnT{j}")
                        nc.scalar.activation(attnT[:], fs["pS"][0:1, :],
                                             ACTF.Sigmoid, bias=s_c0[:], scale=SC)
                        fs["attnT"] = attnT
                    elif st == 14:
                        abc = wp.tile([F, N], bf16, tag="abc", name=f"abc{j}")
                        nc.gpsimd.partition_broadcast(abc[:], fs["attnT"][:])
                        pG = fp.tile([F, N], f32, tag="fps", name=f"pG{j}")
                        nc.tensor.matmul(pG[:], s_fw_vo[:], fs["rlT"][:],
                                         start=True, stop=True)
                        fs.update(abc=abc, pG=pG)
                    elif st == 15:
                        tmpT = wp.tile([F, N], bf16, tag="tmpT", name=f"tmpT{j}")
                        nc.vector.scalar_tensor_tensor(
                            tmpT[:], fs["pG"][:], s_bvo[:], fs["abc"][:],
                            ALU.add, ALU.mult)
                        nrow = min(FT, TS - 1 - tj)
                        nc.vector.tensor_scalar(
                            gbuf[:, (tj + 1) * BS:(tj + 1 + nrow) * BS],
                            tmpT[:, 0:nrow * BS], s_bc_out[:], None, ALU.add)
                        fu_state.pop(j)

                # ---------- recurrence ----------
                GWt, WXt, WHt = W_per[0][0], None, None  # noqa

                def new_ztile(tag, name):
                    """z tile + its single start=True chain head: zeroes all
                    240 cols, deposits L1/L2 biases (start=True on any region
                    resets the whole accumulation bank, so exactly one)."""
                    z = zp.tile([128, C2O], f32, tag=tag, name=name)
                    nc.tensor.matmul(z[:], s_bstk[:], s_bind[:],
                                     start=True, stop=False)
                    return z

                def emit_gates(z, g, s, hcur):
                    """all matmuls for (group g, step s) into z[:, 0:192]."""
                    t0 = min(s, TS - 1)
                    t1 = min(max(s - 1, 0), TS - 1)
                    t2 = min(max(s - 2, 0), TS - 1)
                    cA = t0 * BS + g * G
                    # L0 x-chunks
                    for gi in range(4):
                        a, b_ = blk(0, gi)
                        nc.tensor.matmul(
                            z[:, a:b_], s_w0xa[:, gi * H:(gi + 1) * H],
                            xa_s[:, cA:cA + G], start=False, stop=False)
                        nc.tensor.matmul(
                            z[:, a:b_], s_w0xbb[:, gi * H:(gi + 1) * H],
                            xb_s[:, cA:cA + G], start=False, stop=False)
                    # gammas
                    for l_, tl in ((0, t0), (1, t1), (2, t2)):
                        gw = W_per[l_][0]
                        cG = tl * BS + g * G
                        for gi in range(4):
                            a, b_ = blk(l_, gi)
                            nc.tensor.matmul(
                                z[:, a:b_], gw[:, gi * H:(gi + 1) * H],
                                gbuf[:, cG:cG + G], start=False, stop=False)
                    # h matmuls (critical path: emitted last)
                    for gi in range(4):
                        a, b_ = blk(0, gi)
                        nc.tensor.matmul(
                            z[:, a:b_], s_w0h[:, gi * H:(gi + 1) * H],
                            hcur[:, 0:G], start=False, stop=True)
                    for l_ in (1, 2):
                        wx, wh = W_per[l_][1], W_per[l_][2]
                        hin = hcur[:, (l_ - 1) * G:l_ * G]
                        hown = hcur[:, l_ * G:(l_ + 1) * G]
                        for gi in range(4):
                            a, b_ = blk(l_, gi)
                            nc.tensor.matmul(z[:, a:b_],
                                             wx[:, gi * H:(gi + 1) * H], hin,
                                             start=False, stop=False)
                            nc.tensor.matmul(z[:, a:b_],
                                             wh[:, gi * H:(gi + 1) * H], hown,
                                             start=False, stop=True)

                def s3(t_, gi):
                    """[128, 3, 16] AP over gate gi of all 3 layers."""
                    return t_[:, 0:C2O].rearrange(
                        "p (l g x) -> p l g x", l=3, g=4)[:, :, gi, :]

                def r3(t_):
                    return t_[:].rearrange("p (l x) -> p l x", l=3)

                def emit_sigma(z, g, s):
                    S = wp.tile([128, C2O], bf16, tag=f"S{g}", name=f"S{g}_{s}")
                    nc.scalar.activation(S[:], z[:], ACTF.Sigmoid)
                    if s == 0:
                        nc.vector.memset(S[:, 64:C2O], 0.0)
                    elif s == 1:
                        nc.vector.memset(S[:, 128:C2O], 0.0)
                    return S

                def emit_h(S_own, cnew, g, s):
                    """h(g,s) = sigma_o * tanh(c_new) (own-group, decoupled)."""
                    th = wp.tile([128, 48], bf16, tag="th", name=f"th{g}_{s}",
                                 bufs=4)
                    nc.scalar.activation(th[:], cnew[:], ACTF.Tanh)
                    h = wp.tile([128, 48], bf16, tag=f"h{g}", name=f"h{g}_{s}",
                                bufs=3)
                    nc.vector.tensor_tensor(r3(h), s3(S_own, 2), r3(th), ALU.mult)
                    return h

                def emit_cpath(S, csrc, g, s):
                    """c(g,s) = sigma_i*c_hat + sigma_f*c_old (SBUF f32)."""
                    ct = wp.tile([128, 48], bf16, tag="ct", name=f"ct{g}_{s}",
                                 bufs=4)
                    nc.vector.tensor_scalar(r3(ct), s3(S, 3), 2.0, 1.0,
                                            ALU.mult, ALU.subtract)
                    m1 = wp.tile([128, 48], bf16, tag="m1", name=f"m1{g}_{s}",
                                 bufs=4)
                    nc.vector.tensor_tensor(r3(m1), s3(S, 0), r3(ct), ALU.mult)
                    m2 = wp.tile([128, 48], f32, tag="m2", name=f"m2{g}_{s}",
                                 bufs=4)
                    nc.gpsimd.tensor_tensor(r3(m2), s3(S, 1),
                                            csrc.rearrange("p (l x) -> p l x", l=3),
                                            ALU.mult)
                    cnew = wp.tile([128, 48], f32, tag=f"c{g}",
                                   name=f"c{g}_{s}", bufs=2)
                    nc.gpsimd.tensor_tensor(cnew[:], m1[:], m2[:], ALU.add)
                    return cnew

                # initial state
                hA = cp.tile([128, 48], bf16, tag="hA0")
                nc.vector.memset(hA[:], 0.0)
                hB = cp.tile([128, 48], bf16, tag="hB0")
                nc.vector.memset(hB[:], 0.0)
                cA_src = zero48
                cB_src = zero48

                for k in range(PROP):
                    prepass_a(k)
                    prepass_b(k)
                for j in range(PROF):
                    for st in range(FT):
                        fusion_stage(j, st)

                for s in range(STOT):
                    if s % TB == 0:
                        k = s // TB + PROP
                        if k < NPT:
                            prepass_a(k)
                    elif s % TB == 2:
                        k = s // TB + PROP
                        if k < NPT:
                            prepass_b(k)
                    jf = s // FT + PROF
                    if jf < NFC:
                        fusion_stage(jf, s % FT)

                    # --- group A ---
                    pA = new_ztile("zA", f"zA_{s}")
                    emit_gates(pA, 0, s, hA)
                    SA = emit_sigma(pA, 0, s)
                    if DEBUG_GBUF and s == DEBUG_S:
                        nc.sync.dma_start(dbg3[:], SA[:])
                    cA_src = emit_cpath(SA, cA_src[:], 0, s)
                    hA = emit_h(SA, cA_src, 0, s)
                    # --- group B ---
                    pB = new_ztile("zB", f"zB_{s}")
                    emit_gates(pB, 1, s, hB)
                    SB = emit_sigma(pB, 1, s)
                    cB_src = emit_cpath(SB, cB_src[:], 1, s)
                    hB = emit_h(SB, cB_src, 1, s)

                # ---------- output from final h tiles (l=2 slice) ----------
                for gi_, h2g in ((0, hA[:, 32:48]), (1, hB[:, 32:48])):
                    pog = tp.tile([128, 128], f32, tag="ptx", name=f"po{gi_}")
                    nc.tensor.matmul(pog[0:16, 0:2], h2g, s_regw[:],
                                     start=True, stop=False)
                    nc.tensor.matmul(pog[0:16, 0:2], s_ones32[:, 0:16], s_regb[:],
                                     start=False, stop=True)
                    og = wp.tile([16, 2], f32, tag="outs", name=f"outs{gi_}")
                    nc.scalar.copy(og[:], pog[0:16, 0:2])
                    nc.sync.dma_start(outd[gi_ * 16:(gi_ + 1) * 16, :], og[:])
                if DEBUG_GBUF:
                    nc.sync.dma_start(dbg[:], gbuf[:])
                    nc.sync.dma_start(dbg2[:], xb_s[:])

    nc.compile()
    return nc


def prep_inputs(inputs, t_steps=T):
    TS = t_steps

    def g(k):
        return np.asarray(inputs[k], dtype=np.float32)

    def b(x):
        return np.ascontiguousarray(x.astype(BF))

    perm = np.concatenate([np.arange(0, H), np.arange(H, 2 * H),
                           np.arange(3 * H, 4 * H), np.arange(2 * H, 3 * H)])

    base_w0 = g("base_w0")[:, perm]
    w0h = base_w0[:H].copy()
    w0x = base_w0[H:]
    w0xa = w0x[:DA].copy()
    w0xb = w0x[DA:].copy()
    b0 = g("base_b0")[perm]
    bw12 = g("base_w12")
    w1 = bw12[0][:, perm]
    w2 = bw12[1][:, perm]
    w1h, w1x = w1[:H].copy(), w1[H:].copy()
    w2h, w2x = w2[:H].copy(), w2[H:].copy()
    b12 = g("base_b12")
    b1, b2 = b12[0][perm].copy(), b12[1][perm].copy()

    def gwstack(gw):
        return np.concatenate([gw[0], -LAM * gw[1], gw[3], gw[2]], axis=1)

    gw0 = gwstack(g("gam_w0"))
    gw12 = g("gam_w12")
    gw1, gw2 = gwstack(gw12[0]), gwstack(gw12[1])

    w0xbb = np.concatenate([w0xb, b0[None, :]], axis=0)
    for arr in (w0h, w0xa, w0xbb, gw0, w1h, w1x, gw1, w2h, w2x, gw2, b1, b2):
        arr[..., 3 * H:4 * H] *= 2.0

    # L1/L2 bias stack [8, 128]: rows = (L1 i,f,o,c, L2 i,f,o,c) bias vectors
    bstk = np.stack([b1[i * H:(i + 1) * H] for i in range(4)]
                    + [b2[i * H:(i + 1) * H] for i in range(4)])
    bind = np.zeros((8, 192), np.float32)
    for k in range(8):
        bind[k, 64 + k * 16:64 + (k + 1) * 16] = 1.0

    f_q_w, f_k_w = g("f_q_w"), g("f_k_w")
    f_q_b, f_k_b = g("f_q_b"), g("f_k_b")
    A = f_q_w @ f_k_w.T
    u = f_q_w @ f_k_b
    v = f_k_w @ f_q_b
    c0 = float(f_q_b @ f_k_b)
    fw_vo = g("f_v_w") @ g("f_out_w")
    bvo = g("f_v_b") @ g("f_out_w")

    consts = {
        "w0xa": b(w0xa), "w0xbb": b(w0xbb), "w0h": b(w0h), "gw0": b(gw0),
        "w1x": b(w1x), "w1h": b(w1h), "gw1": b(gw1),
        "w2x": b(w2x), "w2h": b(w2h), "gw2": b(gw2),
        "bstk": b(bstk), "bind": b(bind),
        "fw_amp": g("f_amp_w"), "fw_ph": g("f_ph_w"), "fw_r1": g("f_rlos_w1"),
        "fwg_ph": b(g("f_gate_w")[0:F]), "fwg_am": b(g("f_gate_w")[F:2 * F]),
        "fw_r2": b(g("f_rlos_w2")),
        "fA": b(A.T), "fw_vo": b(fw_vo),
        "ucol": b(u[:, None]), "vcol": b(v[:, None]),
        "bc_amp": g("f_amp_b")[:, None], "bc_ph": g("f_ph_b")[:, None],
        "bc_gate": g("f_gate_b")[:, None], "bc_r1": g("f_rlos_b1")[:, None],
        "bc_r2": g("f_rlos_b2")[:, None],
        "bvo": bvo[:, None].astype(np.float32),
        "bc_out": g("f_out_b")[:, None],
        "c0t": np.full((1, 1), c0, np.float32),
        "identd": np.eye(128, dtype=np.float32),
        "ident2": 2.0 * np.eye(128, dtype=np.float32),
        "onesr": np.ones((1, 1), BF),
        "regw": b(g("reg_w")), "regb": b(g("reg_b")[None, :]),
    }
    consts = {k: np.ascontiguousarray(v) for k, v in consts.items()}

    hrrp = g("hrrp")[:, :TS, :]
    ac = g("amplitude_corr")[:, :TS]
    pc_ = g("phase_corr")[:, :TS]
    rldel = g("rlos_delta")[:, :TS, :]

    in_maps = []
    for c in range(NCORES):
        sl = slice(c * BS, (c + 1) * BS)
        m = dict(consts)
        m["hrrp"] = np.ascontiguousarray(hrrp[sl].reshape(BS * TS, D))
        m["ac"] = np.ascontiguousarray(ac[sl])
        m["pc"] = np.ascontiguousarray(pc_[sl])
        m["rldel"] = np.ascontiguousarray(rldel[sl])
        in_maps.append(m)
    return in_maps


_NC_CACHE = {}


def _get_nc(t_steps=T):
    if t_steps not in _NC_CACHE:
        _NC_CACHE[t_steps] = build_nc(t_steps)
    return _NC_CACHE[t_steps]


def run(inputs, t_steps=T, **kwargs):
    nc = _get_nc(t_steps)
    in_maps = prep_inputs(inputs, t_steps)
    res = run_bass_kernel_spmd(nc, in_maps, core_ids=list(range(NCORES)), **kwargs)
    out = np.concatenate([res.results[c]["out"] for c in range(NCORES)], axis=0)
    return out, res


def kernel(**inputs) -> np.ndarray:
    out, _ = run(inputs)
    return out.astype(np.float32)


# revision 14
# speedup vs baseline: 1.6744x; 1.0027x over previous
"""Trainium2 Bass kernel for nn_CGLSTM (TwoStageFusion + 3-layer gamma-modulated LSTM).

v2: transposed (feature-major) bf16 recurrence.

Sharding: pure data parallel over batch B=256 across 8 NeuronCores (32/core),
split into 2 interleaved groups of 16 to pipeline the serial recurrence.

Per step s and group g, one PSUM tile [128, 240] holds z^T for all 3 wavefront
layers (12 gate-blocks x 16 batch, gate cols pre-scaled so tanh folds into
sigmoid) plus a 48-col region where the *other* group's 2*c state was written
by the vector engines; ONE sigmoid instruction then yields all gates and
tanh(c_other) at once.  Gate matmuls are bf16 with the weights stationary
(out free size 16), x^T and gamma^T live SBUF-resident, so the inner loop
does no DMA and no transposes.
"""

import sys

sys.path.insert(0, "/opt/trn_rl_repo")

import numpy as np  # noqa: E402
import ml_dtypes  # noqa: E402

import concourse.bass as bass  # noqa: E402, F401
import concourse.tile as tile  # noqa: E402
from concourse import bacc, mybir  # noqa: E402
from concourse.bass_utils import run_bass_kernel_spmd  # noqa: E402

f32 = mybir.dt.float32
f32r = mybir.dt.float32r
bf16 = mybir.dt.bfloat16
u32 = mybir.dt.uint32
ACTF = mybir.ActivationFunctionType
ALU = mybir.AluOpType
BF = ml_dtypes.bfloat16

B, T, D, H, F = 256, 512, 200, 128, 128
LAM = 0.5
NCORES = 8
BS = B // NCORES   # 32
G = BS // 2        # 16 per group
DA, DB = 128, D - 128
NG = 4 * H
FT = 16            # fusion chunk t-size (= #stages)
TB = 4             # prepass t's per [128,200] tile
DEBUG_GBUF = False
DEBUG_S = 0
C2O = 192          # c2 region offset in the z psum tile
ZW = C2O + 48      # 240


def _R(t):
    return t[:].bitcast(f32r)


def build_nc(t_steps=T):
    TS = t_steps
    STOT = TS + 2
    nc = bacc.Bacc("TRN2", target_bir_lowering=False, debug=False, num_devices=NCORES)

    def din(name, shape, dt=f32):
        return nc.dram_tensor(name, shape, dt, kind="ExternalInput").ap()

    hrrp = din("hrrp", [BS * TS, D])
    ac = nc.dram_tensor("ac", [BS, TS], f32r, kind="ExternalInput").ap()
    pc = nc.dram_tensor("pc", [BS, TS], f32r, kind="ExternalInput").ap()
    rldel = nc.dram_tensor("rldel", [BS, TS, 2], f32r, kind="ExternalInput").ap()
    # recurrence weights (bf16, gate order [i f o c], c-cols pre-scaled x2)
    w0xa = din("w0xa", [DA, NG], bf16)
    w0xbb = din("w0xbb", [DB + 1, NG], bf16)   # row DB = L0 bias
    w0h = din("w0h", [H, NG], bf16)
    gw0 = din("gw0", [F, NG], bf16)
    w1x = din("w1x", [H, NG], bf16)
    w1h = din("w1h", [H, NG], bf16)
    gw1 = din("gw1", [F, NG], bf16)
    w2x = din("w2x", [H, NG], bf16)
    w2h = din("w2h", [H, NG], bf16)
    gw2 = din("gw2", [F, NG], bf16)
    bstk = din("bstk", [8, 128], bf16)         # L1/L2 biases stacked
    bind = din("bind", [8, C2O], bf16)         # bias block indicator (zero-padded)
    # fusion weights
    fw_amp = din("fw_amp", [1, F])
    fw_ph = din("fw_ph", [1, F])
    fw_r1 = din("fw_r1", [2, F])
    fwg_ph = din("fwg_ph", [F, F], bf16)
    fwg_am = din("fwg_am", [F, F], bf16)
    fw_r2 = din("fw_r2", [F, F], bf16)
    fA = din("fA", [F, F], bf16)               # W_q W_k^T (lhsT layout A^T)
    fw_vo = din("fw_vo", [F, F], bf16)
    ucol = din("ucol", [F, 1], bf16)
    vcol = din("vcol", [F, 1], bf16)
    bc_amp = din("bc_amp", [F, 1])
    bc_ph = din("bc_ph", [F, 1])
    bc_gate = din("bc_gate", [F, 1])
    bc_r1 = din("bc_r1", [F, 1])
    bc_r2 = din("bc_r2", [F, 1])
    bvo = din("bvo", [F, 1])
    bc_out = din("bc_out", [F, 1])
    c0t = din("c0t", [1, 1])
    identd = din("identd", [128, 128])
    ident2 = din("ident2", [128, 128])
    onesr = din("onesr", [1, 1], bf16)
    regw = din("regw", [H, 2], bf16)
    regb = din("regb", [1, 2], bf16)

    outd = nc.dram_tensor("out", [BS, 2], f32, kind="ExternalOutput").ap()
    dbg = nc.dram_tensor("dbg", [F, TS * BS], bf16, kind="ExternalOutput").ap() \
        if DEBUG_GBUF else None
    dbg2 = nc.dram_tensor("dbg2", [DB + 1, TS * BS], bf16,
                          kind="ExternalOutput").ap() if DEBUG_GBUF else None
    dbg3 = nc.dram_tensor("dbg3", [128, C2O], bf16,
                          kind="ExternalOutput").ap() if DEBUG_GBUF else None

    SC = float(F) ** -0.5
    NPT = TS // TB             # prepass tiles
    NFC = (TS + FT - 1) // FT  # fusion chunks
    PROP = min(8, NPT)
    PROF = min(3, NFC)

    # gate-block column helpers: block (l, gi) = cols (l*4+gi)*16
    def blk(l_, gi):
        o = (l_ * 4 + gi) * 16
        return o, o + 16

    with tile.TileContext(nc) as tc:
        with tc.tile_pool(name="const", bufs=1) as cp:
            def load(name, shape, src, dt=bf16):
                t_ = cp.tile(shape, dt, tag=name)
                nc.sync.dma_start(t_[:], src)
                return t_

            s_w0xa = load("s_w0xa", [DA, NG], w0xa[:])
            s_w0xbb = load("s_w0xbb", [DB + 1, NG], w0xbb[:])
            s_w0h = load("s_w0h", [H, NG], w0h[:])
            s_gw0 = load("s_gw0", [F, NG], gw0[:])
            s_w1x = load("s_w1x", [H, NG], w1x[:])
            s_w1h = load("s_w1h", [H, NG], w1h[:])
            s_gw1 = load("s_gw1", [F, NG], gw1[:])
            s_w2x = load("s_w2x", [H, NG], w2x[:])
            s_w2h = load("s_w2h", [H, NG], w2h[:])
            s_gw2 = load("s_gw2", [F, NG], gw2[:])
            s_bstk = load("s_bstk", [8, 128], bstk[:])
            s_bind = load("s_bind", [8, C2O], bind[:])
            def load_r(name, shape, src):
                t_ = cp.tile(shape, f32, tag=name)
                nc.gpsimd.dma_start(_R(t_), src)
                return t_

            s_fw_amp = load_r("s_fw_amp", [1, F], fw_amp[:])
            s_fw_ph = load_r("s_fw_ph", [1, F], fw_ph[:])
            s_fw_r1 = load_r("s_fw_r1", [2, F], fw_r1[:])
            s_fwg_ph = load("s_fwg_ph", [F, F], fwg_ph[:])
            s_fwg_am = load("s_fwg_am", [F, F], fwg_am[:])
            s_fw_r2 = load("s_fw_r2", [F, F], fw_r2[:])
            s_fA = load("s_fA", [F, F], fA[:])
            s_fw_vo = load("s_fw_vo", [F, F], fw_vo[:])
            s_u = load("s_u", [F, 1], ucol[:])
            s_v = load("s_v", [F, 1], vcol[:])
            s_bc_amp = load("s_bc_amp", [F, 1], bc_amp[:], f32)
            s_bc_ph = load("s_bc_ph", [F, 1], bc_ph[:], f32)
            s_bc_gate = load("s_bc_gate", [F, 1], bc_gate[:], f32)
            s_bc_r1 = load("s_bc_r1", [F, 1], bc_r1[:], f32)
            s_bc_r2 = load("s_bc_r2", [F, 1], bc_r2[:], f32)
            s_bvo = load("s_bvo", [F, 1], bvo[:], f32)
            s_bc_out = load("s_bc_out", [F, 1], bc_out[:], f32)
            s_c0 = load("s_c0", [1, 1], c0t[:], f32)
            s_ident = load("s_ident", [128, 128], identd[:], f32)
            s_ident2 = load_r("s_ident2", [128, 128], ident2[:])
            s_regw = load("s_regw", [H, 2], regw[:])
            s_regb = load("s_regb", [1, 2], regb[:])

            s_ones1 = cp.tile([128, 1], bf16, tag="s_ones1")
            nc.vector.memset(s_ones1[:], 1.0)
            s_ones32 = cp.tile([1, 32], bf16, tag="s_ones32")
            nc.vector.memset(s_ones32[:], 1.0)
            zero48 = cp.tile([128, 48], f32, tag="zero48")
            nc.vector.memset(zero48[:], 0.0)

            # SBUF-resident transposed |x| (bf16) and gamma_sh (bf16)
            xa_s = cp.tile([DA, TS * BS], bf16, tag="xa_s")
            xb_s = cp.tile([DB + 1, TS * BS], bf16, tag="xb_s")
            # ones row for L0 bias: DMA broadcast (engine ops need 32-aligned
            # partition starts; DMA does not)
            nc.sync.dma_start(
                xb_s[DB:DB + 1, :].rearrange("p (a b) -> p a b", a=TS * BS),
                bass.AP(tensor=onesr.tensor, offset=0,
                        ap=[[0, 1], [0, TS * BS], [0, 1]]))
            gbuf = cp.tile([F, TS * BS], bf16, tag="gbuf")
            nc.vector.memset(gbuf[:, 0:BS], 0.0)        # gamma_sh[0] = 0

            W_per = {
                0: (s_gw0, None, s_w0h),
                1: (s_gw1, s_w1x, s_w1h),
                2: (s_gw2, s_w2x, s_w2h),
            }

            hr3 = hrrp.rearrange("(b tt) d -> b tt d", b=BS)

            with (
                tc.tile_pool(name="work", bufs=2) as wp,
                tc.tile_pool(name="zps", bufs=2, space="PSUM") as zp,
                tc.tile_pool(name="fps", bufs=2, space="PSUM") as fp,
                tc.tile_pool(name="tps", bufs=1, space="PSUM") as tp,
            ):
                # ---------- prepass (2 stages per tile) ----------
                pre_state = {}

                def prepass_a(k):
                    t0 = k * TB
                    raw = wp.tile([128, D], f32, tag="raw", name=f"raw{k}", bufs=3)
                    nc.sync.dma_start(
                        raw[:], hr3[:, t0:t0 + TB, :].rearrange("b tt d -> tt b d"))
                    ab = wp.tile([128, D], f32, tag="ab", name=f"ab{k}", bufs=3)
                    nc.vector.tensor_scalar(
                        ab[:].bitcast(u32), raw[:].bitcast(u32),
                        0x7FFFFFFF, None, ALU.bitwise_and)
                    pre_state[k] = ab

                def prepass_b(k):
                    ab = pre_state.pop(k)
                    pt1 = tp.tile([128, 128], f32, tag="ptx", name=f"pt1_{k}")
                    nc.tensor.transpose(pt1[0:DA, :], ab[:, 0:DA], s_ident[:, :])
                    pt2 = tp.tile([128, 128], f32, tag="ptx", name=f"pt2_{k}")
                    nc.tensor.transpose(pt2[0:DB, :], ab[:, DA:D], s_ident[:, :])
                    c0_ = k * 128
                    nc.vector.tensor_copy(xa_s[:, c0_:c0_ + 128], pt1[0:DA, :])
                    nc.vector.tensor_copy(xb_s[0:DB, c0_:c0_ + 128], pt2[0:DB, :])

                # ---------- fusion (16 stages per chunk) ----------
                fu_state = {}

                def fusion_stage(j, st):
                    tj = j * FT
                    N = FT * BS
                    fs = fu_state.setdefault(j, {})
                    if st == 0:
                        a_row = wp.tile([1, N], f32, tag="a_row", name=f"a_row{j}")
                        nc.sync.dma_start(
                            a_row[:].bitcast(f32r).rearrange(
                                "p (tt b) -> p tt b", tt=FT),
                            bass.AP(tensor=ac.tensor, offset=tj,
                                    ap=[[0, 1], [1, FT], [TS, BS]]))
                        p_row = wp.tile([1, N], f32, tag="p_row", name=f"p_row{j}")
                        nc.sync.dma_start(
                            p_row[:].bitcast(f32r).rearrange(
                                "p (tt b) -> p tt b", tt=FT),
                            bass.AP(tensor=pc.tensor, offset=tj,
                                    ap=[[0, 1], [1, FT], [TS, BS]]))
                        rl2 = wp.tile([2, N], f32, tag="rl2", name=f"rl2{j}")
                        for c_ in range(2):
                            nc.sync.dma_start(
                                rl2[:].bitcast(f32r)[c_:c_ + 1, :].rearrange(
                                    "p (tt b) -> p tt b", tt=FT),
                                bass.AP(tensor=rldel.tensor, offset=tj * 2 + c_,
                                        ap=[[0, 1], [2, FT], [2 * TS, BS]]))
                        fs.update(a_row=a_row, p_row=p_row, rl2=rl2)
                    elif st == 1:
                        pA = fp.tile([F, N], f32, tag="fps", name=f"pA{j}")
                        nc.tensor.matmul(pA[:], _R(s_fw_amp),
                                         fs["a_row"][:].bitcast(f32r),
                                         start=True, stop=True)
                        pB = fp.tile([F, N], f32, tag="fps", name=f"pB{j}")
                        nc.tensor.matmul(pB[:], _R(s_fw_ph),
                                         fs["p_row"][:].bitcast(f32r),
                                         start=True, stop=True)
                        fs.update(pA=pA, pB=pB)
                    elif st == 2:
                        ampT = wp.tile([F, N], bf16, tag="ampT", name=f"ampT{j}")
                        nc.scalar.activation(ampT[:], fs["pA"][:], ACTF.Tanh,
                                             bias=s_bc_amp[:])
                        fs["ampT"] = ampT
                    elif st == 3:
                        phT = wp.tile([F, N], bf16, tag="phT", name=f"phT{j}")
                        nc.scalar.activation(phT[:], fs["pB"][:], ACTF.Tanh,
                                             bias=s_bc_ph[:])
                        fs["phT"] = phT
                    elif st == 4:
                        pC = fp.tile([F, N], f32, tag="fps", name=f"pC{j}")
                        nc.tensor.matmul(pC[:], s_fwg_ph[:], fs["phT"][:],
                                         start=True, stop=False)
                        nc.tensor.matmul(pC[:], s_fwg_am[:], fs["ampT"][:],
                                         start=False, stop=True)
                        fs["pC"] = pC
                    elif st == 5:
                        betaT = wp.tile([F, N], bf16, tag="betaT", name=f"betaT{j}")
                        nc.scalar.activation(betaT[:], fs["pC"][:], ACTF.Sigmoid,
                                             bias=s_bc_gate[:])
                        fs["betaT"] = betaT
                    elif st == 6:
                        dT = wp.tile([F, N], bf16, tag="dT", name=f"dT{j}")
                        nc.vector.tensor_tensor(dT[:], fs["phT"][:], fs["ampT"][:],
                                                ALU.subtract)
                        mT = wp.tile([F, N], bf16, tag="mT", name=f"mT{j}")
                        nc.vector.tensor_tensor(mT[:], fs["betaT"][:], dT[:],
                                                ALU.mult)
                        corrT = wp.tile([F, N], bf16, tag="corrT", name=f"corrT{j}")
                        nc.vector.tensor_tensor(corrT[:], mT[:], fs["ampT"][:],
                                                ALU.add)
                        fs["corrT"] = corrT
                    elif st == 7:
                        pR1 = fp.tile([F, N], f32, tag="fps", name=f"pR1{j}")
                        nc.tensor.matmul(pR1[:], _R(s_fw_r1),
                                         fs["rl2"][:].bitcast(f32r),
                                         start=True, stop=True)
                        fs["pR1"] = pR1
                    elif st == 8:
                        rl1T = wp.tile([F, N], bf16, tag="rl1T", name=f"rl1T{j}")
                        nc.scalar.activation(rl1T[:], fs["pR1"][:], ACTF.Tanh,
                                             bias=s_bc_r1[:])
                        fs["rl1T"] = rl1T
                    elif st == 9:
                        pR2 = fp.tile([F, N], f32, tag="fps", name=f"pR2{j}")
                        nc.tensor.matmul(pR2[:], s_fw_r2[:], fs["rl1T"][:],
                                         start=True, stop=True)
                        fs["pR2"] = pR2
                    elif st == 10:
                        rlT = wp.tile([F, N], bf16, tag="rlT", name=f"rlT{j}")
                        nc.scalar.activation(rlT[:], fs["pR2"][:], ACTF.Tanh,
                                             bias=s_bc_r2[:])
                        fs["rlT"] = rlT
                    elif st == 11:
                        pAr = fp.tile([F, N], f32, tag="fps", name=f"pAr{j}")
                        nc.tensor.matmul(pAr[:], s_fA[:], fs["rlT"][:],
                                         start=True, stop=True)
                        wT = wp.tile([F, N], bf16, tag="wT", name=f"wT{j}")
                        nc.vector.tensor_tensor(wT[:], fs["corrT"][:], pAr[:],
                                                ALU.mult)
                        fs["wT"] = wT
                    elif st == 12:
                        pS = fp.tile([F, N], f32, tag="fps", name=f"pS{j}")
                        nc.tensor.matmul(pS[0:1, :], s_ones1[:], fs["wT"][:],
                                         start=True, stop=False)
                        nc.tensor.matmul(pS[0:1, :], s_u[:], fs["corrT"][:],
                                         start=False, stop=False)
                        nc.tensor.matmul(pS[0:1, :], s_v[:], fs["rlT"][:],
                                         start=False, stop=True)
                        fs["pS"] = pS
                    elif st == 13:
                        attnT = wp.tile([1, N], bf16, tag="attnT", name=f"attnT{j}")
                        nc.scalar.activation(attnT[:], fs["pS"][0:1, :],
                                             ACTF.Sigmoid, bias=s_c0[:], scale=SC)
                        fs["attnT"] = attnT
                    elif st == 14:
                        abc = wp.tile([F, N], bf16, tag="abc", name=f"abc{j}")
                        nc.gpsimd.partition_broadcast(abc[:], fs["attnT"][:])
                        pG = fp.tile([F, N], f32, tag="fps", name=f"pG{j}")
                        nc.tensor.matmul(pG[:], s_fw_vo[:], fs["rlT"][:],
                                         start=True, stop=True)
                        fs.update(abc=abc, pG=pG)
                    elif st == 15:
                        tmpT = wp.tile([F, N], bf16, tag="tmpT", name=f"tmpT{j}")
                        nc.vector.scalar_tensor_tensor(
                            tmpT[:], fs["pG"][:], s_bvo[:], fs["abc"][:],
                            ALU.add, ALU.mult)
                        nrow = min(FT, TS - 1 - tj)
                        nc.vector.tensor_scalar(
                            gbuf[:, (tj + 1) * BS:(tj + 1 + nrow) * BS],
                            tmpT[:, 0:nrow * BS], s_bc_out[:], None, ALU.add)
                        fu_state.pop(j)

                # ---------- recurrence ----------
                GWt, WXt, WHt = W_per[0][0], None, None  # noqa

                def new_ztile(tag, name):
                    """z tile + its single start=True chain head: zeroes all
                    240 cols, deposits L1/L2 biases (start=True on any region
                    resets the whole accumulation bank, so exactly one)."""
                    z = zp.tile([128, C2O], f32, tag=tag, name=name)
                    nc.tensor.matmul(z[:], s_bstk[:], s_bind[:],
                                     start=True, stop=False)
                    return z

                def emit_gates(z, g, s, hcur):
                    """all matmuls for (group g, step s) into z[:, 0:192]."""
                    t0 = min(s, TS - 1)
                    t1 = min(max(s - 1, 0), TS - 1)
                    t2 = min(max(s - 2, 0), TS - 1)
                    cA = t0 * BS + g * G
                    # L0 x-chunks
                    for gi in range(4):
                        a, b_ = blk(0, gi)
                        nc.tensor.matmul(
                            z[:, a:b_], s_w0xa[:, gi * H:(gi + 1) * H],
                            xa_s[:, cA:cA + G], start=False, stop=False)
                        nc.tensor.matmul(
                            z[:, a:b_], s_w0xbb[:, gi * H:(gi + 1) * H],
                            xb_s[:, cA:cA + G], start=False, stop=False)
                    # gammas
                    for l_, tl in ((0, t0), (1, t1), (2, t2)):
                        gw = W_per[l_][0]
                        cG = tl * BS + g * G
                        for gi in range(4):
                            a, b_ = blk(l_, gi)
                            nc.tensor.matmul(
                                z[:, a:b_], gw[:, gi * H:(gi + 1) * H],
                                gbuf[:, cG:cG + G], start=False, stop=False)
                    # h matmuls (critical path: emitted last)
                    for gi in range(4):
                        a, b_ = blk(0, gi)
                        nc.tensor.matmul(
                            z[:, a:b_], s_w0h[:, gi * H:(gi + 1) * H],
                            hcur[:, 0:G], start=False, stop=True)
                    for l_ in (1, 2):
                        wx, wh = W_per[l_][1], W_per[l_][2]
                        hin = hcur[:, (l_ - 1) * G:l_ * G]
                        hown = hcur[:, l_ * G:(l_ + 1) * G]
                        for gi in range(4):
                            a, b_ = blk(l_, gi)
                            nc.tensor.matmul(z[:, a:b_],
                                             wx[:, gi * H:(gi + 1) * H], hin,
                                             start=False, stop=False)
                            nc.tensor.matmul(z[:, a:b_],
                                             wh[:, gi * H:(gi + 1) * H], hown,
                                             start=False, stop=True)

                def s3(t_, gi):
                    """[128, 3, 16] AP over gate gi of all 3 layers."""
                    return t_[:, 0:C2O].rearrange(
                        "p (l g x) -> p l g x", l=3, g=4)[:, :, gi, :]

                def r3(t_):
                    return t_[:].rearrange("p (l x) -> p l x", l=3)

                def emit_sigma(z, g, s):
                    S = wp.tile([128, C2O], bf16, tag=f"S{g}", name=f"S{g}_{s}")
                    nc.scalar.activation(S[:], z[:], ACTF.Sigmoid)
                    if s == 0:
                        nc.vector.memset(S[:, 64:C2O], 0.0)
                    elif s == 1:
                        nc.vector.memset(S[:, 128:C2O], 0.0)
                    return S

                def emit_h(S_own, cnew, g, s):
                    """h(g,s) = sigma_o * tanh(c_new) (own-group, decoupled)."""
                    th = wp.tile([128, 48], bf16, tag="th", name=f"th{g}_{s}",
                                 bufs=4)
                    nc.scalar.activation(th[:], cnew[:], ACTF.Tanh)
                    h = wp.tile([128, 48], bf16, tag=f"h{g}", name=f"h{g}_{s}",
                                bufs=3)
                    nc.vector.tensor_tensor(r3(h), s3(S_own, 2), r3(th), ALU.mult)
                    return h

                def emit_cpath(S, csrc, g, s):
                    """c(g,s) = sigma_i*c_hat + sigma_f*c_old (SBUF f32)."""
                    ct = wp.tile([128, 48], bf16, tag="ct", name=f"ct{g}_{s}",
                                 bufs=4)
                    nc.vector.tensor_scalar(r3(ct), s3(S, 3), 2.0, 1.0,
                                            ALU.mult, ALU.subtract)
                    m1 = wp.tile([128, 48], bf16, tag="m1", name=f"m1{g}_{s}",
                                 bufs=4)
                    nc.vector.tensor_tensor(r3(m1), s3(S, 0), r3(ct), ALU.mult)
                    m2 = wp.tile([128, 48], f32, tag="m2", name=f"m2{g}_{s}",
                                 bufs=4)
                    nc.gpsimd.tensor_tensor(r3(m2), s3(S, 1),
                                            csrc.rearrange("p (l x) -> p l x", l=3),
                                            ALU.mult)
                    cnew = wp.tile([128, 48], f32, tag=f"c{g}",
                                   name=f"c{g}_{s}", bufs=2)
                    nc.gpsimd.tensor_tensor(cnew[:], m1[:], m2[:], ALU.add)
                    return cnew

                # initial state
                hA = cp.tile([128, 48], bf16, tag="hA0")
                nc.vector.memset(hA[:], 0.0)
                hB = cp.tile([128, 48], bf16, tag="hB0")
                nc.vector.memset(hB[:], 0.0)
                cA_src = zero48
                cB_src = zero48

                for k in range(PROP):
                    prepass_a(k)
                    prepass_b(k)
                for j in range(PROF):
                    for st in range(FT):
                        fusion_stage(j, st)

                for s in range(STOT):
                    if s % TB == 0:
                        k = s // TB + PROP
                        if k < NPT:
                            prepass_a(k)
                    elif s % TB == 2:
                        k = s // TB + PROP
                        if k < NPT:
                            prepass_b(k)
                    jf = s // FT + PROF
                    if jf < NFC:
                        fusion_stage(jf, s % FT)

                    # A gates+sigma, A c-path, then B gates+sigma BEFORE
                    # tanh(c_A): keeps the in-order Act queue free of
                    # head-of-line blocking on the A c-path.
                    pA = new_ztile("zA", f"zA_{s}")
                    emit_gates(pA, 0, s, hA)
                    SA = emit_sigma(pA, 0, s)
                    if DEBUG_GBUF and s == DEBUG_S:
                        nc.sync.dma_start(dbg3[:], SA[:])
                    cA_src = emit_cpath(SA, cA_src[:], 0, s)
                    pB = new_ztile("zB", f"zB_{s}")
                    emit_gates(pB, 1, s, hB)
                    SB = emit_sigma(pB, 1, s)
                    hA = emit_h(SA, cA_src, 0, s)
                    cB_src = emit_cpath(SB, cB_src[:], 1, s)
                    hB = emit_h(SB, cB_src, 1, s)

                # ---------- output from final h tiles (l=2 slice) ----------
                for gi_, h2g in ((0, hA[:, 32:48]), (1, hB[:, 32:48])):
                    pog = tp.tile([128, 128], f32, tag="ptx", name=f"po{gi_}")
                    nc.tensor.matmul(pog[0:16, 0:2], h2g, s_regw[:],
                                     start=True, stop=False)
                    nc.tensor.matmul(pog[0:16, 0:2], s_ones32[:, 0:16], s_regb[:],
                                     start=False, stop=True)
                    og = wp.tile([16, 2], f32, tag="outs", name=f"outs{gi_}")
                    nc.scalar.copy(og[:], pog[0:16, 0:2])
                    nc.sync.dma_start(outd[gi_ * 16:(gi_ + 1) * 16, :], og[:])
                if DEBUG_GBUF:
                    nc.sync.dma_start(dbg[:], gbuf[:])
                    nc.sync.dma_start(dbg2[:], xb_s[:])

    nc.compile()
    return nc


def prep_inputs(inputs, t_steps=T):
    TS = t_steps

    def g(k):
        return np.asarray(inputs[k], dtype=np.float32)

    def b(x):
        return np.ascontiguousarray(x.astype(BF))

    perm = np.concatenate([np.arange(0, H), np.arange(H, 2 * H),
                           np.arange(3 * H, 4 * H), np.arange(2 * H, 3 * H)])

    base_w0 = g("base_w0")[:, perm]
    w0h = base_w0[:H].copy()
    w0x = base_w0[H:]
    w0xa = w0x[:DA].copy()
    w0xb = w0x[DA:].copy()
    b0 = g("base_b0")[perm]
    bw12 = g("base_w12")
    w1 = bw12[0][:, perm]
    w2 = bw12[1][:, perm]
    w1h, w1x = w1[:H].copy(), w1[H:].copy()
    w2h, w2x = w2[:H].copy(), w2[H:].copy()
    b12 = g("base_b12")
    b1, b2 = b12[0][perm].copy(), b12[1][perm].copy()

    def gwstack(gw):
        return np.concatenate([gw[0], -LAM * gw[1], gw[3], gw[2]], axis=1)

    gw0 = gwstack(g("gam_w0"))
    gw12 = g("gam_w12")
    gw1, gw2 = gwstack(gw12[0]), gwstack(gw12[1])

    w0xbb = np.concatenate([w0xb, b0[None, :]], axis=0)
    for arr in (w0h, w0xa, w0xbb, gw0, w1h, w1x, gw1, w2h, w2x, gw2, b1, b2):
        arr[..., 3 * H:4 * H] *= 2.0

    # L1/L2 bias stack [8, 128]: rows = (L1 i,f,o,c, L2 i,f,o,c) bias vectors
    bstk = np.stack([b1[i * H:(i + 1) * H] for i in range(4)]
                    + [b2[i * H:(i + 1) * H] for i in range(4)])
    bind = np.zeros((8, 192), np.float32)
    for k in range(8):
        bind[k, 64 + k * 16:64 + (k + 1) * 16] = 1.0

    f_q_w, f_k_w = g("f_q_w"), g("f_k_w")
    f_q_b, f_k_b = g("f_q_b"), g("f_k_b")
    A = f_q_w @ f_k_w.T
    u = f_q_w @ f_k_b
    v = f_k_w @ f_q_b
    c0 = float(f_q_b @ f_k_b)
    fw_vo = g("f_v_w") @ g("f_out_w")
    bvo = g("f_v_b") @ g("f_out_w")

    consts = {
        "w0xa": b(w0xa), "w0xbb": b(w0xbb), "w0h": b(w0h), "gw0": b(gw0),
        "w1x": b(w1x), "w1h": b(w1h), "gw1": b(gw1),
        "w2x": b(w2x), "w2h": b(w2h), "gw2": b(gw2),
        "bstk": b(bstk), "bind": b(bind),
        "fw_amp": g("f_amp_w"), "fw_ph": g("f_ph_w"), "fw_r1": g("f_rlos_w1"),
        "fwg_ph": b(g("f_gate_w")[0:F]), "fwg_am": b(g("f_gate_w")[F:2 * F]),
        "fw_r2": b(g("f_rlos_w2")),
        "fA": b(A.T), "fw_vo": b(fw_vo),
        "ucol": b(u[:, None]), "vcol": b(v[:, None]),
        "bc_amp": g("f_amp_b")[:, None], "bc_ph": g("f_ph_b")[:, None],
        "bc_gate": g("f_gate_b")[:, None], "bc_r1": g("f_rlos_b1")[:, None],
        "bc_r2": g("f_rlos_b2")[:, None],
        "bvo": bvo[:, None].astype(np.float32),
        "bc_out": g("f_out_b")[:, None],
        "c0t": np.full((1, 1), c0, np.float32),
        "identd": np.eye(128, dtype=np.float32),
        "ident2": 2.0 * np.eye(128, dtype=np.float32),
        "onesr": np.ones((1, 1), BF),
        "regw": b(g("reg_w")), "regb": b(g("reg_b")[None, :]),
    }
    consts = {k: np.ascontiguousarray(v) for k, v in consts.items()}

    hrrp = g("hrrp")[:, :TS, :]
    ac = g("amplitude_corr")[:, :TS]
    pc_ = g("phase_corr")[:, :TS]
    rldel = g("rlos_delta")[:, :TS, :]

    in_maps = []
    for c in range(NCORES):
        sl = slice(c * BS, (c + 1) * BS)
        m = dict(consts)
        m["hrrp"] = np.ascontiguousarray(hrrp[sl].reshape(BS * TS, D))
        m["ac"] = np.ascontiguousarray(ac[sl])
        m["pc"] = np.ascontiguousarray(pc_[sl])
        m["rldel"] = np.ascontiguousarray(rldel[sl])
        in_maps.append(m)
    return in_maps


_NC_CACHE = {}


def _get_nc(t_steps=T):
    if t_steps not in _NC_CACHE:
        _NC_CACHE[t_steps] = build_nc(t_steps)
    return _NC_CACHE[t_steps]


def run(inputs, t_steps=T, **kwargs):
    nc = _get_nc(t_steps)
    in_maps = prep_inputs(inputs, t_steps)
    res = run_bass_kernel_spmd(nc, in_maps, core_ids=list(range(NCORES)), **kwargs)
    out = np.concatenate([res.results[c]["out"] for c in range(NCORES)], axis=0)
    return out, res


def kernel(**inputs) -> np.ndarray:
    out, _ = run(inputs)
    return out.astype(np.float32)
